# revision 1
# baseline (speedup 1.0000x reference)
"""Trainium2 Bass kernel for nn_BNet (hypergraph GNN message passing), 8 cores.

Strategy
--------
All node/hyperedge intermediates live in degree-grouped, permuted order (the
final output is a (1,1) scalar after global pooling, so order is free):

- Nodes and hyperedges with degree>=1 are dealt round-robin per degree class
  across the 8 cores, tiled 128 at a time, one degree per tile. Segment
  reductions become fixed-width strided reduces; 1/deg becomes a
  compile-time scalar per tile.
- Phase B: xl = h @ W1 + b1 and u = xl @ att1 per 128-node tile (PE), xl
  shards AllGather'd into a full table.
- Phase A: per-hyperedge mean of gathered xl rows + projected pin-feature
  sums -> e_feat (+ v = e_feat @ att2), AllGather'd.
- Phase C: gather e_feat rows per pin (indirect DMA, batched 128 slots per
  partition), softmax attention per node, PNA (mean/max/min/std) via strided
  reduces, Wpost via PE transpose + matmul, pooled sums accumulated in PSUM,
  AllReduce, tiny MLP head.

Pins are gathered via indirect DMA at ~300 GB/s/core; padded slots point at
a guaranteed all-zero table row (no OOB, no masking needed).
"""

import numpy as np

import bass_rust
import concourse.bass as bass
import concourse.tile as tile
from concourse import mybir
from concourse.bass_utils import run_bass_kernel_spmd
from concourse.masks import make_identity
from concourse.vector_clock import ScopedClock

# ----------------------------------------------------------------- constants
N_NODES = 200000
N_HE = 100000
NNZ = 1000000
F_IN = 32  # 29 + 2 + 1
C = 64
NCORES = 8
P = 128
W_EF = 65  # e_feat row: 64 dims + v
K_CH = 128  # gather slots per partition per chunk
MAX_RUN = 8  # tiles per batched run (PSUM free-dim limit 512 = 8*64)
SLOPE = 0.1
F32 = mybir.dt.float32
I32 = mybir.dt.int32
AX = mybir.AxisListType.X
AF = mybir.ActivationFunctionType


# ------------------------------------------------------- walrus workarounds
def _patched_drain_and_barrier(self, tick_clock, wait_clock):
    nc = self.nc
    assert self.sems is not None
    handles = list(self.sems.allocated().values())
    scratch = nc.sync.sem_inc(handles[0], 0) if handles else nc.sync.drain()
    wait_clock.add_sem_waits(scratch.ins, ScopedClock({None: tick_clock.global_clock}))
    waits = list(scratch.ins.sync_info.on_wait)
    scratch.ins.sync_info = bass_rust.SyncInfo(on_wait=[], on_update=[])
    by_name = {h.name: h for h in handles}
    for w in waits:
        nc.sync.wait_ge(by_name[w.ant_name], w.wait_value)
    nc.sync.drain()
    nc.all_engine_barrier()
    popped = nc._tile_sem_poison_stack.pop()
    assert popped is self._sem_poison
    nc.clear_and_free_semaphores(handles)
    nc.all_engine_barrier()


tile.TileContext._drain_and_barrier = _patched_drain_and_barrier

_WS_CTR = [0]


def _split_waits(nc):
    """This walrus build allows at most one sync-wait per instruction; hoist
    extras onto NoOps inserted just before, same engine."""
    for fn in nc.m.functions:
        for bb in fn.blocks:
            insts = list(bb.instructions)
            new = []
            for inst in insts:
                si = inst.sync_info
                if si is not None and len(si.on_wait) > 1:
                    waits = list(si.on_wait)
                    for w in waits[:-1]:
                        _WS_CTR[0] += 1
                        new.append(
                            mybir.InstNoOp(
                                name=f"waitsplit_{_WS_CTR[0]}",
                                engine=inst.engine,
                                sync_info=mybir.SyncInfo(on_wait=[w], on_update=[]),
                                bass_nofuse=True,
                            )
                        )
                    inst.sync_info = mybir.SyncInfo(
                        on_wait=[waits[-1]], on_update=list(si.on_update)
                    )
                new.append(inst)
            bb.instructions = new


# ----------------------------------------------------------- preprocessing
def _partition_by_degree(deg, ncores):
    """Deal ids with deg>=1 round-robin per degree class across cores.

    Returns (core, local_row, tiles, n_rows) where tiles is the common
    per-core tile list [(degree, base_row)] and n_rows includes one final
    all-dummy tile (guaranteed zero rows; last row is the gather sentinel).
    """
    n = len(deg)
    order = np.lexsort((np.arange(n), deg))
    order = order[deg[order] >= 1]
    d_sorted = deg[order].astype(np.int64)
    # group boundaries by degree
    change = np.nonzero(np.diff(d_sorted))[0] + 1
    starts = np.r_[0, change]
    ends = np.r_[change, len(order)]
    rank = np.arange(len(order)) - np.repeat(starts, ends - starts)
    core_of = (rank % ncores).astype(np.int32)
    lrank = rank // ncores
    tiles = []
    local = np.zeros(len(order), np.int64)
    base = 0
    for s, e in zip(starts, ends):
        d = int(d_sorted[s])
        m = int(np.ceil((e - s) / ncores))  # max per-core count
        t_d = int(np.ceil(m / P))
        idx = slice(s, e)
        local[idx] = base + lrank[idx]
        for t in range(t_d):
            tiles.append((d, base + t * P))
        base += t_d * P
    n_rows = base + P  # one extra all-dummy tile
    core = np.full(n, -1, np.int32)
    loc = np.full(n, -1, np.int64)
    core[order] = core_of
    loc[order] = local
    return core, loc, tiles, n_rows


def _pack_chunks(tiles):
    """Pack tiles into K_CH-slot chunks; a tile never crosses a chunk.
    Returns (placement [(chunk, col)], n_chunks, runs) with runs =
    [(chunk, col0, degree, [tile_indices])] capped at MAX_RUN tiles."""
    place = []
    chunk, cur = 0, 0
    used = {}
    for d, _ in tiles:
        if cur + d > K_CH:
            chunk += 1
            cur = 0
        place.append((chunk, cur))
        cur += d
        used[chunk] = cur
    n_chunks = chunk + 1
    runs = []
    i = 0
    while i < len(tiles):
        d = tiles[i][0]
        ch, col = place[i]
        j = i
        while (
            j + 1 < len(tiles)
            and tiles[j + 1][0] == d
            and place[j + 1][0] == ch
            and j + 1 - i + 1 <= MAX_RUN
        ):
            j += 1
        runs.append((ch, col, d, list(range(i, j + 1))))
        i = j + 1
    return place, n_chunks, runs, used


def _rank_within(seg_ids):
    """j-rank of each element within its segment (stable, segment-sorted)."""
    order = np.argsort(seg_ids, kind="stable")
    sorted_ids = seg_ids[order]
    change = np.nonzero(np.diff(sorted_ids))[0] + 1
    starts = np.r_[0, change]
    counts = np.diff(np.r_[starts, len(sorted_ids)])
    r = np.arange(len(sorted_ids)) - np.repeat(starts, counts)
    out = np.empty(len(seg_ids), np.int64)
    out[order] = r
    return out


def preprocess(inputs):
    x = np.asarray(inputs["x"])
    fake_pos = np.asarray(inputs["fake_pos"])
    edge_index = np.asarray(inputs["edge_index"])
    pin_feature = np.asarray(inputs["pin_feature"])
    macro_index = np.asarray(inputs["macro_index"])
    node_idx = edge_index[0].astype(np.int64)
    he_idx = edge_index[1].astype(np.int64)

    deg_n = np.bincount(node_idx, minlength=N_NODES)
    deg_e = np.bincount(he_idx, minlength=N_HE)

    core_n, loc_n, tiles_n, NRN = _partition_by_degree(deg_n, NCORES)
    core_e, loc_e, tiles_e, NRE = _partition_by_degree(deg_e, NCORES)
    placeA, nchA, runsA, usedA = _pack_chunks(tiles_e)
    placeC, nchC, runsC, usedC = _pack_chunks(tiles_n)

    xrow = core_n.astype(np.int64) * NRN + loc_n  # xl_full row per node
    erow = core_e.astype(np.int64) * NRE + loc_e  # ef_full row per hyperedge
    sent_x = NRN - 1
    sent_e = NRE - 1

    # --- per-pin placement, hyperedge-major (phase A) ---
    jA = _rank_within(he_idx)
    cA = core_e[he_idx]
    tA = loc_e[he_idx] // P
    pA = loc_e[he_idx] % P
    chA = np.array([pl[0] for pl in placeA], np.int64)[tA]
    colA = np.array([pl[1] for pl in placeA], np.int64)[tA] + jA

    # --- node-major (phase C) ---
    jC = _rank_within(node_idx)
    cC = core_n[node_idx]
    tC = loc_n[node_idx] // P
    pC = loc_n[node_idx] % P
    chC = np.array([pl[0] for pl in placeC], np.int64)[tC]
    colC = np.array([pl[1] for pl in placeC], np.int64)[tC] + jC

    # ismacro / macro multiplicity
    ismacro = np.zeros(N_NODES, np.float32)
    ismacro[macro_index] = 1.0
    mult = np.bincount(macro_index, minlength=N_NODES).astype(np.float32)

    TN = len(tiles_n)
    h_full = np.concatenate([x, fake_pos, ismacro[:, None]], 1)  # (N, 32)

    per_core = []
    for c in range(NCORES):
        # phase A index + pin features
        aidx = np.full((nchA, P, K_CH), sent_x, np.int32)
        pf = np.zeros((nchA, P, K_CH, 4), np.float32)
        m = cA == c
        aidx[chA[m], pA[m], colA[m]] = xrow[node_idx[m]].astype(np.int32)
        pf[chA[m], pA[m], colA[m]] = pin_feature[m]

        # phase C index
        cidx = np.full((nchC, P, K_CH), sent_e, np.int32)
        m2 = cC == c
        cidx[chC[m2], pC[m2], colC[m2]] = erow[he_idx[m2]].astype(np.int32)

        # hT (33, NRN): features transposed, ones row, dummies zero
        hT = np.zeros((F_IN + 1, NRN), np.float32)
        sel = core_n == c
        hT[:F_IN, loc_n[sel]] = h_full[sel].T
        hT[F_IN, loc_n[sel]] = 1.0

        # pooling weights [128, TN*2]
        wpool = np.zeros((P, TN * 2), np.float32)
        nl = loc_n[sel]
        wpool[nl % P, (nl // P) * 2] = mult[sel]
        wpool[nl % P, (nl // P) * 2 + 1] = 1.0

        per_core.append(dict(aidx=aidx, pf=pf, cidx=cidx, hT=hT, wpool=wpool))

    return dict(
        per_core=per_core,
        tiles_n=tiles_n,
        tiles_e=tiles_e,
        runsA=runsA,
        runsC=runsC,
        usedA=usedA,
        usedC=usedC,
        nchA=nchA,
        nchC=nchC,
        NRN=NRN,
        NRE=NRE,
        core_n=core_n,
        loc_n=loc_n,
        core_e=core_e,
        loc_e=loc_e,
    )



def _const_col_factory(nc, pool):
    cache = {}
    def cc(val, parts=P):
        key = (float(val), parts)
        if key not in cache:
            t = pool.tile([parts, 1], F32, tag=f"cc{len(cache)}")
            nc.vector.memset(t[:], float(val))
            cache[key] = t
        return cache[key][:]
    return cc

# ----------------------------------------------------------- device program
def build_program(prep, inputs, debug=False):
    NRN, NRE = prep["NRN"], prep["NRE"]
    nchA, nchC = prep["nchA"], prep["nchC"]
    runsA, runsC = prep["runsA"], prep["runsC"]
    TN = len(prep["tiles_n"])
    TBN = NRN // P  # phase B tiles (incl final dummy tile)
    core_ids = list(range(NCORES))

    nc = bass.Bass("TRN2", target_bir_lowering=False, debug=False, num_devices=NCORES)

    # inputs
    hT_in = nc.declare_dram_parameter("hT", [F_IN + 1, NRN], F32, isOutput=False)
    W1b_in = nc.declare_dram_parameter("W1b", [F_IN + 1, C], F32, isOutput=False)
    att1_in = nc.declare_dram_parameter("att1", [C, 1], F32, isOutput=False)
    att2r_in = nc.declare_dram_parameter("att2r", [P, C], F32, isOutput=False)
    Wpin_in = nc.declare_dram_parameter("Wpin", [4, C], F32, isOutput=False)
    aidx_in = nc.declare_dram_parameter("aidx", [nchA, P, K_CH], I32, isOutput=False)
    pf_in = nc.declare_dram_parameter("pf", [nchA, P, K_CH * 4], F32, isOutput=False)
    cidx_in = nc.declare_dram_parameter("cidx", [nchC, P, K_CH], I32, isOutput=False)
    wpool_in = nc.declare_dram_parameter("wpool", [P, TN * 2], F32, isOutput=False)
    WpostT_in = nc.declare_dram_parameter("WpostT", [4 * C, C], F32, isOutput=False)
    bpost_in = nc.declare_dram_parameter("bpost", [1, C], F32, isOutput=False)
    Wm1_in = nc.declare_dram_parameter("Wm1", [2 * C, C], F32, isOutput=False)
    bm1_in = nc.declare_dram_parameter("bm1", [1, C], F32, isOutput=False)
    Wm2_in = nc.declare_dram_parameter("Wm2", [C, C // 2], F32, isOutput=False)
    bm2_in = nc.declare_dram_parameter("bm2", [1, C // 2], F32, isOutput=False)
    Wm3_in = nc.declare_dram_parameter("Wm3", [C // 2, 1], F32, isOutput=False)
    bm3_in = nc.declare_dram_parameter("bm3", [1, 1], F32, isOutput=False)
    z_out = nc.declare_dram_parameter("z", [1, 1], F32, isOutput=True)
    if debug:
        xl_dbg = nc.declare_dram_parameter("xl_dbg", [NRN, C], F32, isOutput=True)
        ef_dbg = nc.declare_dram_parameter("ef_dbg", [NRE, W_EF], F32, isOutput=True)
        u_dbg = nc.declare_dram_parameter("u_dbg", [P, NRN // P], F32, isOutput=True)
        stg_dbg = nc.declare_dram_parameter("stg_dbg", [P, K_CH * C], F32, isOutput=True)
        xs_dbg = nc.declare_dram_parameter("xs_dbg", [P, MAX_RUN * C], F32, isOutput=True)
        pool_dbg = nc.declare_dram_parameter("pool_dbg", [2, C], F32, isOutput=True)

    # internal DRAM
    xl_shard = nc.dram_tensor("xl_shard", [NRN, C], F32)
    xl_full = nc.dram_tensor("xl_full", [NCORES * NRN, C], F32, addr_space="Shared")
    ef_shard = nc.dram_tensor("ef_shard", [NRE, W_EF], F32)
    ef_full = nc.dram_tensor("ef_full", [NCORES * NRE, W_EF], F32, addr_space="Shared")
    ar_in = nc.dram_tensor("ar_in", [2, C], F32)
    ar_out = nc.dram_tensor("ar_out", [2, C], F32, addr_space="Shared")

    # ---------------- context 1: weight fold + phase B (xl, u) -------------
    u_sb = nc.alloc_sbuf_tensor("u_sb", [P, TBN], F32)  # persistent u columns
    with tile.TileContext(nc) as tc:
        with (
            tc.tile_pool(name="cpool", bufs=1) as cpool,
            tc.tile_pool(name="bpool", bufs=4) as bpool,
            tc.tile_pool(name="bpsum", bufs=1, space="PSUM") as bpsum,
            tc.tile_pool(name="xpsum", bufs=4, space="PSUM") as xpsum,
        ):
            ident = cpool.tile([P, P], F32)
            make_identity(nc, ident[:])
            w1b = cpool.tile([F_IN + 1, C], F32)
            nc.sync.dma_start(out=w1b[:], in_=W1b_in[:, :])
            att1 = cpool.tile([C, 1], F32)
            nc.sync.dma_start(out=att1[:], in_=att1_in[:, :])
            # rhs2 = [W1b | W1b@att1]
            w1bT_ps = bpsum.tile([P, P], F32, space="PSUM")
            nc.tensor.transpose(
                out=w1bT_ps[:C, : F_IN + 1], in_=w1b[:], identity=ident[: F_IN + 1, : F_IN + 1]
            )
            w1bT = cpool.tile([C, F_IN + 1], F32)
            nc.vector.tensor_copy(w1bT[:], w1bT_ps[:C, : F_IN + 1])
            rhs2 = cpool.tile([F_IN + 1, C + 1], F32)
            nc.vector.tensor_copy(rhs2[:, :C], w1b[:])
            uw_ps = bpsum.tile([F_IN + 1, 2], F32, space="PSUM")
            nc.tensor.matmul(
                uw_ps[:, :1], lhsT=w1bT[:], rhs=att1[:], start=True, stop=True
            )
            nc.vector.tensor_copy(rhs2[:, C : C + 1], uw_ps[:, :1])

            for t in range(TBN):
                ht = bpool.tile([F_IN + 1, P], F32)
                nc.sync.dma_start(out=ht[:], in_=hT_in[:, t * P : (t + 1) * P])
                xlu_ps = xpsum.tile([P, C + 1], F32, space="PSUM")
                nc.tensor.matmul(
                    xlu_ps[:], lhsT=ht[:], rhs=rhs2[:], start=True, stop=True
                )
                xlt = bpool.tile([P, C], F32)
                nc.scalar.activation(xlt[:], xlu_ps[:, :C], AF.Copy)
                nc.scalar.activation(u_sb[:, t : t + 1], xlu_ps[:, C : C + 1], AF.Copy)
                nc.sync.dma_start(out=xl_shard[t * P : (t + 1) * P, :], in_=xlt[:])

    # AllGather xl
    with (
        nc.semaphore("ag1_sem") as ag1_sem,
        nc.Block() as blk,
    ):

        @blk.gpsimd
        def _(g):
            g.collective_compute(
                "AllGather",
                mybir.AluOpType.bypass,
                replica_groups=[core_ids],
                ins=[xl_shard[:, :]],
                outs=[xl_full[:, :]],
            ).then_inc(ag1_sem, 1)
            g.wait_ge(ag1_sem, 1)

    # ---------------- context 2: phase A (e_feat) --------------------------
    with tile.TileContext(nc) as tc:
        with (
            tc.tile_pool(name="acons", bufs=1) as acons,
            tc.tile_pool(name="aidxp", bufs=3) as aidxp,
            tc.tile_pool(name="astg", bufs=3) as astg,
            tc.tile_pool(name="apfp", bufs=2) as apfp,
            tc.tile_pool(name="arun", bufs=3) as arun,
            tc.tile_pool(name="apsum", bufs=2, space="PSUM") as apsum,
            tc.tile_pool(name="apsum2", bufs=3, space="PSUM") as apsum2,
        ):
            identA = acons.tile([P, P], F32)
            make_identity(nc, identA[:])
            wpin = acons.tile([4, C], F32)
            nc.sync.dma_start(out=wpin[:], in_=Wpin_in[:, :])
            att2r = acons.tile([P, C], F32)
            nc.sync.dma_start(out=att2r[:], in_=att2r_in[:, :])
            ccA = _const_col_factory(nc, acons)
            degsA = sorted({r[2] for r in runsA})
            wpin_d = {}
            for d in degsA:
                wt = acons.tile([4, C], F32, tag=f"wpin{d}")
                nc.scalar.activation(wt[:], wpin[:], AF.Copy, scale=ccA(1.0 / d, 4))
                wpin_d[d] = wt

            # zero the final dummy tile of ef_shard (holds the gather sentinel row)
            zt = acons.tile([P, W_EF], F32)
            nc.vector.memset(zt[:], 0.0)
            TH = len(prep["tiles_e"])
            nc.sync.dma_start(out=ef_shard[TH * P : (TH + 1) * P, :], in_=zt[:])

            run_by_chunk = {}
            for r in runsA:
                run_by_chunk.setdefault(r[0], []).append(r)

            for ch in range(nchA):
                it = aidxp.tile([P, K_CH], I32)
                nc.sync.dma_start(out=it[:], in_=aidx_in[ch])
                stg = astg.tile([P, K_CH * C], F32)
                for j in range(prep["usedA"][ch]):
                    nc.gpsimd.indirect_dma_start(
                        out=stg[:, j * C : (j + 1) * C],
                        out_offset=None,
                        in_=xl_full[:, :],
                        in_offset=bass.IndirectOffsetOnAxis(ap=it[:, j : j + 1], axis=0),
                    )
                pft = apfp.tile([P, K_CH * 4], F32)
                nc.sync.dma_start(out=pft[:], in_=pf_in[ch])
                if debug and ch == 0:
                    nc.sync.dma_start(out=stg_dbg[:, :], in_=stg[:])

                for (_, col, d, tl) in run_by_chunk.get(ch, []):
                    T = len(tl)
                    acc = apsum.tile([P, MAX_RUN * C], F32, space="PSUM", tag="acc")
                    # xl sums -> SBUF, scaled by 1/d on ACT
                    xs = arun.tile([P, MAX_RUN * C], F32, tag="xs")
                    nc.vector.reduce_sum(
                        xs[:, : T * C].rearrange("p (t d) -> p t d", t=T),
                        stg[:, col * C : (col + T * d) * C].rearrange(
                            "p (t j d) -> p t d j", t=T, j=d, d=C
                        ),
                        axis=AX,
                    )
                    nc.scalar.activation(
                        xs[:, : T * C], xs[:, : T * C], AF.Copy, scale=ccA(1.0 / d)
                    )
                    if debug and ch == 0 and d == 2:
                        nc.sync.dma_start(out=xs_dbg[:, : T * C], in_=xs[:, : T * C])
                    # pin feature sums
                    pfs = arun.tile([P, MAX_RUN * 4], F32, tag="pfs")
                    nc.vector.reduce_sum(
                        pfs[:, : T * 4].rearrange("p (t f) -> p t f", t=T),
                        pft[:, col * 4 : (col + T * d) * 4].rearrange(
                            "p (t j f) -> p t f j", t=T, j=d, f=4
                        ),
                        axis=AX,
                    )
                    for ti in range(T):
                        pfT_ps = apsum2.tile([P, P], F32, space="PSUM", tag="pfT")
                        nc.tensor.transpose(
                            out=pfT_ps[:4, :],
                            in_=pfs[:, ti * 4 : (ti + 1) * 4],
                            identity=identA[:],
                        )
                        pfT = arun.tile([4, P], F32, tag="pfTs")
                        nc.scalar.activation(pfT[:], pfT_ps[:4, :], AF.Copy)
                        nc.tensor.matmul(
                            acc[:, ti * C : (ti + 1) * C],
                            lhsT=pfT[:],
                            rhs=wpin_d[d][:],
                            start=True,
                            stop=True,
                        )
                    # ef = xs/d + pf_sum @ (Wpin/d), into 65-stride ef buffer
                    efb = arun.tile([P, MAX_RUN * W_EF], F32, tag="efb")
                    nc.vector.tensor_tensor(
                        out=efb[:, : T * W_EF].rearrange("p (t w) -> p t w", t=T)[
                            :, :, :C
                        ],
                        in0=xs[:, : T * C].rearrange("p (t d) -> p t d", t=T),
                        in1=acc[:, : T * C].rearrange("p (t d) -> p t d", t=T),
                        op=mybir.AluOpType.add,
                    )
                    # v = ef @ att2
                    vt = arun.tile([P, MAX_RUN * C], F32, tag="vt")
                    nc.vector.tensor_tensor(
                        out=vt[:, : T * C],
                        in0=efb[:, : T * W_EF].rearrange("p (t w) -> p t w", t=T)[
                            :, :, :C
                        ],
                        in1=att2r[:, None, :].to_broadcast([P, T, C]),
                        op=mybir.AluOpType.mult,
                    )
                    nc.vector.reduce_sum(
                        efb[:, : T * W_EF].rearrange("p (t w) -> p t w", t=T)[:, :, C],
                        vt[:, : T * C].rearrange("p (t d) -> p t d", t=T),
                        axis=AX,
                    )
                    t0 = tl[0]
                    nc.sync.dma_start(
                        out=ef_shard[t0 * P : (t0 + T) * P, :].rearrange(
                            "(t p) w -> p t w", p=P
                        ),
                        in_=efb[:, : T * W_EF].rearrange("p (t w) -> p t w", t=T),
                    )

    # AllGather ef
    with (
        nc.semaphore("ag2_sem") as ag2_sem,
        nc.Block() as blk2,
    ):

        @blk2.gpsimd
        def _(g):
            g.collective_compute(
                "AllGather",
                mybir.AluOpType.bypass,
                replica_groups=[core_ids],
                ins=[ef_shard[:, :]],
                outs=[ef_full[:, :]],
            ).then_inc(ag2_sem, 1)
            g.wait_ge(ag2_sem, 1)

    # ---------------- context 3: phase C (attention + PNA + pooling) -------
    with tile.TileContext(nc) as tc:
        with (
            tc.tile_pool(name="ccons", bufs=1) as ccons,
            tc.tile_pool(name="cidxp", bufs=3) as cidxp,
            tc.tile_pool(name="cstg", bufs=2) as cstg,
            tc.tile_pool(name="cmsg", bufs=1) as cmsg,
            tc.tile_pool(name="csml", bufs=3) as csml,
            tc.tile_pool(name="cpost", bufs=2) as cpost,
            tc.tile_pool(name="chx", bufs=3) as chx,
            tc.tile_pool(name="cppsum", bufs=1, space="PSUM") as cppsum,
            tc.tile_pool(name="ctpsum", bufs=2, space="PSUM") as ctpsum,
        ):
            identC = ccons.tile([P, P], F32)
            make_identity(nc, identC[:])
            wpostT_a = ccons.tile([P, C], F32)
            nc.sync.dma_start(out=wpostT_a[:], in_=WpostT_in[:P, :])
            wpostT_b = ccons.tile([P, C], F32)
            nc.sync.dma_start(out=wpostT_b[:], in_=WpostT_in[P:, :])
            bpost = ccons.tile([1, C], F32)
            nc.sync.dma_start(out=bpost[:], in_=bpost_in[:, :])
            ones1 = ccons.tile([1, P], F32)
            nc.vector.memset(ones1[:], 1.0)
            wpool = ccons.tile([P, TN * 2], F32)
            nc.sync.dma_start(out=wpool[:], in_=wpool_in[:, :])
            pool_ps = cppsum.tile([2, C], F32, space="PSUM")
            ccC = _const_col_factory(nc, ccons)

            run_by_chunkC = {}
            for r in runsC:
                run_by_chunkC.setdefault(r[0], []).append(r)

            first_mm = [True]
            n_tiles_done = [0]
            for ch in range(nchC):
                it = cidxp.tile([P, K_CH], I32)
                nc.sync.dma_start(out=it[:], in_=cidx_in[ch])
                stg = cstg.tile([P, K_CH * W_EF], F32)
                for j in range(prep["usedC"][ch]):
                    nc.gpsimd.indirect_dma_start(
                        out=stg[:, j * W_EF : (j + 1) * W_EF],
                        out_offset=None,
                        in_=ef_full[:, :],
                        in_offset=bass.IndirectOffsetOnAxis(ap=it[:, j : j + 1], axis=0),
                    )
                for (_, col, d, tl) in run_by_chunkC.get(ch, []):
                    T = len(tl)
                    F = T * d
                    t0 = tl[0]
                    stg_run = stg[:, col * W_EF : (col + F) * W_EF]
                    ef_ap = stg_run.rearrange("p (s w) -> p s w", s=F)[:, :, :C]
                    v_sc = stg_run.rearrange("p (s w) -> p s w", s=F)[:, :, C]

                    # a = lrelu(u + v) ; ex = exp(a)
                    asl = csml.tile([P, K_CH], F32, tag="asl")
                    nc.vector.tensor_tensor(
                        out=asl[:, :F].rearrange("p (t j) -> p t j", t=T),
                        in0=u_sb[:, t0 : t0 + T, None].to_broadcast([P, T, d]),
                        in1=v_sc.rearrange("p (t j) -> p t j", t=T),
                        op=mybir.AluOpType.add,
                    )
                    a2 = csml.tile([P, K_CH], F32, tag="a2")
                    nc.vector.tensor_scalar_mul(a2[:, :F], asl[:, :F], SLOPE)
                    nc.vector.tensor_tensor(
                        out=asl[:, :F],
                        in0=asl[:, :F],
                        in1=a2[:, :F],
                        op=mybir.AluOpType.max,
                    )
                    ex = csml.tile([P, K_CH], F32, tag="ex")
                    nc.scalar.activation(ex[:, :F], asl[:, :F], AF.Exp)
                    den = csml.tile([P, MAX_RUN], F32, tag="den")
                    nc.vector.reduce_sum(
                        den[:, :T],
                        ex[:, :F].rearrange("p (t j) -> p t j", t=T),
                        axis=AX,
                    )
                    nc.vector.reciprocal(den[:, :T], den[:, :T])
                    alpha = csml.tile([P, K_CH], F32, tag="alpha")
                    nc.vector.tensor_tensor(
                        out=alpha[:, :F].rearrange("p (t j) -> p t j", t=T),
                        in0=ex[:, :F].rearrange("p (t j) -> p t j", t=T),
                        in1=den[:, :T, None].to_broadcast([P, T, d]),
                        op=mybir.AluOpType.mult,
                    )
                    # msg = alpha * e_g
                    msg = cmsg.tile([P, K_CH * C], F32, tag="msg")
                    nc.vector.tensor_tensor(
                        out=msg[:, : F * C].rearrange("p (s d) -> p s d", s=F),
                        in0=ef_ap,
                        in1=alpha[:, :F, None].to_broadcast([P, F, C]),
                        op=mybir.AluOpType.mult,
                    )
                    msg3 = msg[:, : F * C].rearrange(
                        "p (t j d) -> p t d j", t=T, j=d, d=C
                    )
                    # pna = [mean | mx | mn | std] per tile, 256 wide
                    post = cpost.tile([P, MAX_RUN * 4 * C], F32, tag="post")
                    post3 = post[:, : T * 4 * C].rearrange(
                        "p (t q d) -> p t q d", t=T, q=4
                    )
                    nc.vector.reduce_max(post3[:, :, 1], msg3, axis=AX)
                    nc.vector.tensor_reduce(
                        post3[:, :, 2], msg3, op=mybir.AluOpType.min, axis=AX
                    )
                    sm = cpost.tile([P, MAX_RUN * C], F32, tag="sm")
                    nc.vector.reduce_sum(
                        sm[:, : T * C].rearrange("p (t d) -> p t d", t=T), msg3, axis=AX
                    )
                    nc.scalar.activation(
                        post3[:, :, 0],
                        sm[:, : T * C].rearrange("p (t d) -> p t d", t=T),
                        AF.Copy,
                        scale=ccC(1.0 / d),
                    )
                    # std = sqrt(max(sq/d - mean^2, 0) + 1e-12)
                    msq = cmsg.tile([P, K_CH * C], F32, tag="msq")
                    nc.scalar.activation(
                        msq[:, : F * C], msg[:, : F * C], AF.Square
                    )
                    sq = cpost.tile([P, MAX_RUN * C], F32, tag="sq")
                    nc.vector.reduce_sum(
                        sq[:, : T * C].rearrange("p (t d) -> p t d", t=T),
                        msq[:, : F * C].rearrange("p (t j d) -> p t d j", t=T, j=d),
                        axis=AX,
                    )
                    m2 = cpost.tile([P, MAX_RUN * C], F32, tag="m2")
                    nc.vector.tensor_tensor(
                        out=m2[:, : T * C].rearrange("p (t d) -> p t d", t=T),
                        in0=post3[:, :, 0],
                        in1=post3[:, :, 0],
                        op=mybir.AluOpType.mult,
                    )
                    nc.vector.tensor_scalar(
                        out=sq[:, : T * C],
                        in0=sq[:, : T * C],
                        scalar1=1.0 / d,
                        scalar2=None,
                        op0=mybir.AluOpType.mult,
                    )
                    nc.vector.tensor_tensor(
                        out=sq[:, : T * C],
                        in0=sq[:, : T * C],
                        in1=m2[:, : T * C],
                        op=mybir.AluOpType.subtract,
                    )
                    nc.vector.tensor_scalar_max(sq[:, : T * C], sq[:, : T * C], 0.0)
                    nc.scalar.activation(
                        post3[:, :, 3],
                        sq[:, : T * C].rearrange("p (t d) -> p t d", t=T),
                        AF.Sqrt,
                        bias=ccC(1e-12),
                    )
                    # hx = lrelu(pna @ Wpost + bpost) per tile, then pool matmul
                    for ti in range(T):
                        t = t0 + ti
                        pn = post[:, ti * 4 * C : (ti + 1) * 4 * C]
                        pT_ps = ctpsum.tile([P, P], F32, space="PSUM", tag="pT")
                        nc.tensor.transpose(
                            out=pT_ps[:], in_=pn[:, :P], identity=identC[:]
                        )
                        pT = chx.tile([P, 2 * P], F32, tag="pT_sb")
                        nc.scalar.activation(pT[:, :P], pT_ps[:], AF.Copy)
                        pT_ps2 = ctpsum.tile([P, P], F32, space="PSUM", tag="pT2")
                        nc.tensor.transpose(
                            out=pT_ps2[:], in_=pn[:, P:], identity=identC[:]
                        )
                        nc.scalar.activation(pT[:, P:], pT_ps2[:], AF.Copy)
                        hx_ps = ctpsum.tile([P, C], F32, space="PSUM", tag="hx")
                        nc.tensor.matmul(
                            hx_ps[:], lhsT=pT[:, :P], rhs=wpostT_a[:],
                            start=True, stop=False,
                        )
                        nc.tensor.matmul(
                            hx_ps[:], lhsT=pT[:, P:], rhs=wpostT_b[:],
                            start=False, stop=False,
                        )
                        nc.tensor.matmul(
                            hx_ps[:],
                            lhsT=ones1[:],
                            rhs=bpost[:],
                            start=False,
                            stop=True,
                        )
                        hx = chx.tile([P, C], F32, tag="hx_sb")
                        hxm = chx.tile([P, C], F32, tag="hxm_sb")
                        nc.scalar.activation(
                            hxm[:], hx_ps[:], AF.Copy, scale=ccC(SLOPE)
                        )
                        nc.vector.tensor_tensor(
                            out=hx[:], in0=hx_ps[:], in1=hxm[:],
                            op=mybir.AluOpType.max,
                        )
                        n_tiles_done[0] += 1
                        nc.tensor.matmul(
                            pool_ps[:],
                            lhsT=wpool[:, 2 * t : 2 * t + 2],
                            rhs=hx[:],
                            start=first_mm[0],
                            stop=(n_tiles_done[0] == TN),
                        )
                        first_mm[0] = False

            pool_sb = ccons.tile([2, C], F32)
            nc.vector.tensor_copy(pool_sb[:], pool_ps[:])
            nc.sync.dma_start(out=ar_in[:, :], in_=pool_sb[:])

    if debug:
        with (
            nc.semaphore("dbg_sem") as dbg_sem,
            nc.Block() as blkd,
        ):

            @blkd.gpsimd
            def _(g):
                g.dma_start(out=xl_dbg[:, :], in_=xl_shard[:, :]).then_inc(dbg_sem, 16)
                g.dma_start(out=ef_dbg[:, :], in_=ef_shard[:, :]).then_inc(dbg_sem, 16)
                g.dma_start(out=pool_dbg[:, :], in_=ar_in[:, :]).then_inc(dbg_sem, 16)
                g.wait_ge(dbg_sem, 48)

        with tile.TileContext(nc) as tc:
            with tc.tile_pool(name="dbgp", bufs=1) as dbgp:
                ut = dbgp.tile([P, NRN // P], F32)
                nc.vector.tensor_copy(ut[:], u_sb[:])
                nc.sync.dma_start(out=u_dbg[:, :], in_=ut[:])

    # AllReduce pooled partials
    with (
        nc.semaphore("ar_sem") as ar_sem,
        nc.Block() as blk3,
    ):

        @blk3.gpsimd
        def _(g):
            g.collective_compute(
                "AllReduce",
                mybir.AluOpType.add,
                replica_groups=[core_ids],
                ins=[ar_in[:, :]],
                outs=[ar_out[:, :]],
            ).then_inc(ar_sem, 1)
            g.wait_ge(ar_sem, 1)

    # ---------------- context 4: MLP head ---------------------------------
    with tile.TileContext(nc) as tc:
        with (
            tc.tile_pool(name="mpool", bufs=1) as mpool,
            tc.tile_pool(name="mpsum", bufs=1, space="PSUM") as mpsum,
        ):
            identM = mpool.tile([P, P], F32)
            make_identity(nc, identM[:])
            onesM = mpool.tile([1, 1], F32)
            nc.vector.memset(onesM[:], 1.0)
            ccM = _const_col_factory(nc, mpool)
            pool2 = mpool.tile([2, C], F32)
            nc.sync.dma_start(out=pool2[:], in_=ar_out[:, :])
            poolT_ps = mpsum.tile([P, P], F32, space="PSUM")
            nc.tensor.transpose(out=poolT_ps[:C, :2], in_=pool2[:], identity=identM[:2, :2])
            pooled = mpool.tile([P, 1], F32)
            nc.scalar.activation(
                pooled[:C, :], poolT_ps[:C, :1], AF.Copy, scale=ccM(1.0 / 512.0, C)
            )
            nc.scalar.activation(
                pooled[C:, :], poolT_ps[:C, 1:2], AF.Copy, scale=ccM(1.0 / N_NODES, C)
            )
            wm1 = mpool.tile([2 * C, C], F32)
            nc.sync.dma_start(out=wm1[:], in_=Wm1_in[:, :])
            bm1 = mpool.tile([1, C], F32)
            nc.sync.dma_start(out=bm1[:], in_=bm1_in[:, :])
            wm2 = mpool.tile([C, C // 2], F32)
            nc.sync.dma_start(out=wm2[:], in_=Wm2_in[:, :])
            bm2 = mpool.tile([1, C // 2], F32)
            nc.sync.dma_start(out=bm2[:], in_=bm2_in[:, :])
            wm3 = mpool.tile([C // 2, 1], F32)
            nc.sync.dma_start(out=wm3[:], in_=Wm3_in[:, :])
            bm3 = mpool.tile([1, 1], F32)
            nc.sync.dma_start(out=bm3[:], in_=bm3_in[:, :])

            def _lrelu_row(dst, src_ps, width):
                tmp = mpool.tile([1, width], F32, tag=f"lr{width}")
                nc.scalar.activation(tmp[:], src_ps[:], AF.Copy, scale=ccM(SLOPE, 1))
                nc.vector.tensor_tensor(
                    out=dst[:], in0=src_ps[:], in1=tmp[:], op=mybir.AluOpType.max
                )

            z1_ps = mpsum.tile([1, C], F32, space="PSUM")
            nc.tensor.matmul(z1_ps[:], lhsT=pooled[:], rhs=wm1[:], start=True, stop=False)
            nc.tensor.matmul(
                z1_ps[:], lhsT=onesM[:].to_broadcast([1, 1]), rhs=bm1[:],
                start=False, stop=True,
            )
            z1 = mpool.tile([1, C], F32)
            _lrelu_row(z1, z1_ps, C)
            z1T_ps = mpsum.tile([P, P], F32, space="PSUM")
            nc.tensor.transpose(out=z1T_ps[:C, :1], in_=z1[:], identity=identM[:1, :1])
            z1T = mpool.tile([C, 1], F32)
            nc.vector.tensor_copy(z1T[:], z1T_ps[:C, :1])
            z2_ps = mpsum.tile([1, C // 2], F32, space="PSUM")
            nc.tensor.matmul(z2_ps[:], lhsT=z1T[:], rhs=wm2[:], start=True, stop=False)
            nc.tensor.matmul(
                z2_ps[:], lhsT=onesM[:].to_broadcast([1, 1]), rhs=bm2[:],
                start=False, stop=True,
            )
            z2 = mpool.tile([1, C // 2], F32)
            _lrelu_row(z2, z2_ps, C // 2)
            z2T_ps = mpsum.tile([P, P], F32, space="PSUM")
            nc.tensor.transpose(out=z2T_ps[: C // 2, :1], in_=z2[:], identity=identM[:1, :1])
            z2T = mpool.tile([C // 2, 1], F32)
            nc.vector.tensor_copy(z2T[:], z2T_ps[: C // 2, :1])
            z3_ps = mpsum.tile([1, 1], F32, space="PSUM")
            nc.tensor.matmul(z3_ps[:], lhsT=z2T[:], rhs=wm3[:], start=True, stop=False)
            nc.tensor.matmul(
                z3_ps[:], lhsT=onesM[:].to_broadcast([1, 1]), rhs=bm3[:],
                start=False, stop=True,
            )
            z3 = mpool.tile([1, 1], F32)
            nc.vector.tensor_copy(z3[:], z3_ps[:])
            nc.sync.dma_start(out=z_out[:, :], in_=z3[:])

    _split_waits(nc)
    return nc


def make_in_maps(prep, inputs):
    W1 = np.asarray(inputs["W1"], np.float32)
    b1 = np.asarray(inputs["b1"], np.float32)
    att = np.asarray(inputs["att"], np.float32)
    Wpost = np.asarray(inputs["Wpost"], np.float32)
    in_maps = []
    for c in range(NCORES):
        pc = prep["per_core"][c]
        in_maps.append(
            dict(
                hT=pc["hT"],
                W1b=np.vstack([W1, b1[None, :]]).astype(np.float32),
                att1=att[:C, None].copy(),
                att2r=np.repeat(att[None, C:], P, 0).copy(),
                Wpin=np.asarray(inputs["Wpin"], np.float32),
                aidx=pc["aidx"],
                pf=pc["pf"].reshape(prep["nchA"], P, K_CH * 4),
                cidx=pc["cidx"],
                wpool=pc["wpool"],
                WpostT=Wpost.astype(np.float32),
                bpost=np.asarray(inputs["bpost"], np.float32)[None, :],
                Wm1=np.asarray(inputs["Wm1"], np.float32),
                bm1=np.asarray(inputs["bm1"], np.float32)[None, :],
                Wm2=np.asarray(inputs["Wm2"], np.float32),
                bm2=np.asarray(inputs["bm2"], np.float32)[None, :],
                Wm3=np.asarray(inputs["Wm3"], np.float32),
                bm3=np.asarray(inputs["bm3"], np.float32)[None, :],
            )
        )
    return in_maps



def _install_ntff_hook():
    """Register the NTFF profile hook trn_boot skips when antenv.axon_hooks is
    absent, so run_bass_kernel_spmd(trace=True) can report exec_time_ns."""
    import sys
    import types

    try:
        if "antenv.axon_hooks" not in sys.modules:
            import antenv

            mod = types.ModuleType("antenv.axon_hooks")
            holder = [None]
            mod.set_axon_ntff_profile_hook = lambda h: holder.__setitem__(0, h)
            mod.get_axon_ntff_profile_hook = lambda: holder[0]
            mod._holder = holder
            sys.modules["antenv.axon_hooks"] = mod
            antenv.axon_hooks = mod
        mod = sys.modules["antenv.axon_hooks"]
        if mod.get_axon_ntff_profile_hook() is None:
            from trn_agent_boot.trn_boot import _ntff_profile_via_ctypes

            mod.set_axon_ntff_profile_hook(
                _ntff_profile_via_ctypes("/opt/axon/libaxon_pjrt.so")
            )
        return mod.get_axon_ntff_profile_hook() is not None
    except Exception:
        return False


_LAST = {}


def kernel(**inputs):
    prep = preprocess(inputs)
    nc = build_program(prep, inputs)
    in_maps = make_in_maps(prep, inputs)
    trace_ok = _install_ntff_hook()
    try:
        res = run_bass_kernel_spmd(
            nc, in_maps, list(range(NCORES)), trace=trace_ok, trace_cores=[0]
        )
    except Exception:
        res = run_bass_kernel_spmd(nc, in_maps, list(range(NCORES)))
    _LAST["res"] = res
    return res.results[0]["z"].astype(np.float32)



# revision 11
# speedup vs baseline: 1.2351x; 1.2351x over previous
"""Trainium2 Bass kernel for nn_BNet (hypergraph GNN message passing), 8 cores.

v2 design (vs v1: no xl table, no xl AllGather, no per-pin INDIRECT1D)
----------------------------------------------------------------------
- Host stages per-pin raw features hpin = [h(32) | pin_feature(4)] in
  hyperedge-major slot layout (bf16).  Because e_feat is linear in the
  per-pin features, phase A computes each 128-hyperedge tile as
  (sum_j hpin_j) @ [W1;Wpin;b1]/d with one PE transpose + one matmul per
  tile; weight column 64 holds (.)@att2 so the matmul emits the full
  65-wide row (e_feat | v) at once.
- e_feat rows stored bf16, padded to 128 ch (256B rows) in a shard table;
  AllGather builds the global table (8*NRE rows, ~51.7k pairs < 65536).
- Phase C gathers per-pin e_feat rows with TWO windowed dma_gather custom
  instructions per 4096-slot chunk (int16 indices address 512B row-PAIRS
  at stride 512B; out-of-window slots read a guaranteed-zero row from the
  shard's dummy tile; windows merged with one bf16 add; the even/odd
  sub-row select is folded into the alpha multiply).
- Per-node softmax drops the max-subtraction (a_raw is O(5); exp is safe
  in fp32 and the subtraction cancels exactly in alpha).
- PNA (mean/max/min/std), Wpost, pooling, AllReduce and the MLP head are
  unchanged from v1.
"""

import ml_dtypes
import numpy as np

import bass_rust
import concourse.bass as bass
import concourse.tile as tile
from concourse import library_config, library_overlay, mybir
from concourse.bass_utils import run_bass_kernel_spmd
from concourse.masks import make_identity
from concourse.vector_clock import ScopedClock

# ----------------------------------------------------------------- constants
N_NODES = 200000
N_HE = 100000
NNZ = 1000000
F_IN = 32  # 29 + 2 + 1
FP = 36  # h(32) + pin_feature(4)
C = 64
NCORES = 8
P = 128
W_EF = 65  # e_feat row: 64 dims + v
KA = 128  # phase A chunk columns
KC = 32  # phase C chunk columns (4096 slots per chunk)
WIN = 32768  # int16 window size in pair rows
MAX_RUN = 8
SLOPE = 0.1
F32 = mybir.dt.float32
BF16 = mybir.dt.bfloat16
I16 = mybir.dt.int16
AX = mybir.AxisListType.X
AF = mybir.ActivationFunctionType
BF = ml_dtypes.bfloat16


# ------------------------------------------------------- walrus workarounds
def _patched_drain_and_barrier(self, tick_clock, wait_clock):
    nc = self.nc
    assert self.sems is not None
    handles = list(self.sems.allocated().values())
    scratch = nc.sync.sem_inc(handles[0], 0) if handles else nc.sync.drain()
    wait_clock.add_sem_waits(scratch.ins, ScopedClock({None: tick_clock.global_clock}))
    waits = list(scratch.ins.sync_info.on_wait)
    scratch.ins.sync_info = bass_rust.SyncInfo(on_wait=[], on_update=[])
    by_name = {h.name: h for h in handles}
    for w in waits:
        nc.sync.wait_ge(by_name[w.ant_name], w.wait_value)
    nc.sync.drain()
    nc.all_engine_barrier()
    popped = nc._tile_sem_poison_stack.pop()
    assert popped is self._sem_poison
    nc.clear_and_free_semaphores(handles)
    nc.all_engine_barrier()


tile.TileContext._drain_and_barrier = _patched_drain_and_barrier

_WS_CTR = [0]


def _split_waits(nc):
    """This walrus build allows at most one sync-wait per instruction; hoist
    extras onto NoOps inserted just before, same engine."""
    for fn in nc.m.functions:
        for bb in fn.blocks:
            insts = list(bb.instructions)
            new = []
            for inst in insts:
                si = inst.sync_info
                if si is not None and len(si.on_wait) > 1:
                    waits = list(si.on_wait)
                    for w in waits[:-1]:
                        _WS_CTR[0] += 1
                        new.append(
                            mybir.InstNoOp(
                                name=f"waitsplit_{_WS_CTR[0]}",
                                engine=inst.engine,
                                sync_info=mybir.SyncInfo(on_wait=[w], on_update=[]),
                                bass_nofuse=True,
                            )
                        )
                    inst.sync_info = mybir.SyncInfo(
                        on_wait=[waits[-1]], on_update=list(si.on_update)
                    )
                new.append(inst)
            bb.instructions = new


# ----------------------------------------------------------- preprocessing
def _partition_by_degree(deg, ncores):
    """Deal ids with deg>=1 round-robin per degree class across cores.

    Returns (core, local_row, tiles, n_rows); tiles is the common per-core
    tile list [(degree, base_row)]; n_rows includes one final all-dummy tile
    (guaranteed-zero rows; used as the window sentinel on the hyperedge side).
    """
    n = len(deg)
    order = np.lexsort((np.arange(n), deg))
    order = order[deg[order] >= 1]
    d_sorted = deg[order].astype(np.int64)
    change = np.nonzero(np.diff(d_sorted))[0] + 1
    starts = np.r_[0, change]
    ends = np.r_[change, len(order)]
    rank = np.arange(len(order)) - np.repeat(starts, ends - starts)
    core_of = (rank % ncores).astype(np.int32)
    lrank = rank // ncores
    tiles = []
    local = np.zeros(len(order), np.int64)
    base = 0
    for s, e in zip(starts, ends):
        d = int(d_sorted[s])
        m = int(np.ceil((e - s) / ncores))
        t_d = int(np.ceil(m / P))
        idx = slice(s, e)
        local[idx] = base + lrank[idx]
        for t in range(t_d):
            tiles.append((d, base + t * P))
        base += t_d * P
    n_rows = base + P
    core = np.full(n, -1, np.int32)
    loc = np.full(n, -1, np.int64)
    core[order] = core_of
    loc[order] = local
    return core, loc, tiles, n_rows


def _pack_chunks(tiles, kch):
    """Pack tiles into kch-slot chunks; a tile never crosses a chunk.
    Returns (placement [(chunk, col)], n_chunks, runs, used) with runs =
    [(chunk, col0, degree, [tile_indices])] capped at MAX_RUN tiles."""
    place = []
    chunk, cur = 0, 0
    used = {}
    for d, _ in tiles:
        assert d <= kch, f"degree {d} exceeds chunk width {kch}"
        if cur + d > kch:
            chunk += 1
            cur = 0
        place.append((chunk, cur))
        cur += d
        used[chunk] = cur
    n_chunks = chunk + 1
    runs = []
    i = 0
    while i < len(tiles):
        d = tiles[i][0]
        ch, col = place[i]
        j = i
        while (
            j + 1 < len(tiles)
            and tiles[j + 1][0] == d
            and place[j + 1][0] == ch
            and j + 1 - i + 1 <= MAX_RUN
        ):
            j += 1
        runs.append((ch, col, d, list(range(i, j + 1))))
        i = j + 1
    return place, n_chunks, runs, used


def _rank_within(seg_ids):
    order = np.argsort(seg_ids, kind="stable")
    sorted_ids = seg_ids[order]
    change = np.nonzero(np.diff(sorted_ids))[0] + 1
    starts = np.r_[0, change]
    counts = np.diff(np.r_[starts, len(sorted_ids)])
    r = np.arange(len(sorted_ids)) - np.repeat(starts, counts)
    out = np.empty(len(seg_ids), np.int64)
    out[order] = r
    return out


def preprocess(inputs):
    x = np.asarray(inputs["x"], np.float32)
    fake_pos = np.asarray(inputs["fake_pos"], np.float32)
    edge_index = np.asarray(inputs["edge_index"])
    pin_feature = np.asarray(inputs["pin_feature"], np.float32)
    macro_index = np.asarray(inputs["macro_index"])
    node_idx = edge_index[0].astype(np.int64)
    he_idx = edge_index[1].astype(np.int64)

    deg_n = np.bincount(node_idx, minlength=N_NODES)
    deg_e = np.bincount(he_idx, minlength=N_HE)

    core_n, loc_n, tiles_n, NRN = _partition_by_degree(deg_n, NCORES)
    core_e, loc_e, tiles_e, NRE = _partition_by_degree(deg_e, NCORES)
    placeA, nchA, runsA, usedA = _pack_chunks(tiles_e, KA)
    placeC, nchC, runsC, usedC = _pack_chunks(tiles_n, KC)

    NPAIR = NCORES * (NRE // 2)
    assert NRE % 2 == 0 and NPAIR <= 2 * WIN, (NRE, NPAIR)
    TH = len(tiles_e)
    z_pair_loc = (TH * P) // 2  # first pair row of the all-dummy (zero) tile
    Z0 = 0 * (NRE // 2) + z_pair_loc
    Z1 = (NCORES - 1) * (NRE // 2) + z_pair_loc
    assert Z0 < WIN <= Z1 and Z1 - WIN <= WIN - 1, (Z0, Z1)

    grow_pair = (core_e.astype(np.int64) * NRE + loc_e) // 2
    parity = loc_e % 2

    # per-pin placement: hyperedge-major (phase A)
    jA = _rank_within(he_idx)
    cA = core_e[he_idx]
    tA = loc_e[he_idx] // P
    pA = loc_e[he_idx] % P
    chA = np.array([pl[0] for pl in placeA], np.int64)[tA]
    colA = np.array([pl[1] for pl in placeA], np.int64)[tA] + jA

    # node-major (phase C)
    jC = _rank_within(node_idx)
    cC = core_n[node_idx]
    tC = loc_n[node_idx] // P
    pC = loc_n[node_idx] % P
    chC = np.array([pl[0] for pl in placeC], np.int64)[tC]
    colC = np.array([pl[1] for pl in placeC], np.int64)[tC] + jC

    ismacro = np.zeros(N_NODES, np.float32)
    ismacro[macro_index] = 1.0
    mult = np.bincount(macro_index, minlength=N_NODES).astype(np.float32)

    TN = len(tiles_n)
    h_full = np.concatenate([x, fake_pos, ismacro[:, None]], 1)  # (N, 32)
    hp_full = np.concatenate([h_full[node_idx], pin_feature], 1)  # (NNZ, 36)

    per_core = []
    for c in range(NCORES):
        m = cA == c
        hpin = np.zeros((nchA, P, KA, FP), np.float32)
        hpin[chA[m], pA[m], colA[m]] = hp_full[m]
        hpin_bf = np.ascontiguousarray(
            hpin.astype(BF).reshape(nchA, P, KA * FP)
        )

        # phase C windowed gather indices + parity masks
        m2 = cC == c
        gp = np.full((nchC, P, KC), Z0, np.int64)
        par = np.zeros((nchC, P, KC), np.int64)
        live = np.zeros((nchC, P, KC), bool)
        gp[chC[m2], pC[m2], colC[m2]] = grow_pair[he_idx[m2]]
        par[chC[m2], pC[m2], colC[m2]] = parity[he_idx[m2]]
        live[chC[m2], pC[m2], colC[m2]] = True

        idx0 = np.where(gp < WIN, gp, Z0)
        idx1 = np.where(gp >= WIN, gp - WIN, Z1 - WIN)

        def wrapc(a):
            # slot (p, col) -> linear i = col*128 + p -> [16, NI/16] x8 groups
            out = np.empty((nchC, P, KC * 8), np.int16)
            for ch in range(nchC):
                lin = a[ch].T.reshape(-1)  # i = col*128 + p
                m16 = lin.reshape(-1, 16).T  # [16, NI/16]
                out[ch] = np.tile(m16, (8, 1))
            return out

        cidx0 = wrapc(idx0)
        cidx1 = wrapc(idx1)
        maskE = (live & (par == 0)).astype(np.float32)
        maskO = (live & (par == 1)).astype(np.float32)
        cmask = np.ascontiguousarray(
            np.concatenate([maskE, maskO], axis=2)
        )  # [nchC, P, 2*KC] f32

        # hT (33, NRN): own nodes' features transposed + ones row
        hT = np.zeros((F_IN + 1, NRN), np.float32)
        sel = core_n == c
        hT[:F_IN, loc_n[sel]] = h_full[sel].T
        hT[F_IN, loc_n[sel]] = 1.0

        # pooling weights [128, TN*2]
        wpool = np.zeros((P, TN * 2), np.float32)
        nl = loc_n[sel]
        wpool[nl % P, (nl // P) * 2] = mult[sel]
        wpool[nl % P, (nl // P) * 2 + 1] = 1.0

        per_core.append(
            dict(
                hpin=hpin_bf,
                cidx0=cidx0,
                cidx1=cidx1,
                cmask=cmask,
                hT=hT,
                wpool=wpool,
                dbg=dict(gp=gp, par=par, live=live),
            )
        )

    return dict(
        per_core=per_core,
        tiles_n=tiles_n,
        tiles_e=tiles_e,
        runsA=runsA,
        runsC=runsC,
        usedA=usedA,
        usedC=usedC,
        nchA=nchA,
        nchC=nchC,
        NRN=NRN,
        NRE=NRE,
        NPAIR=NPAIR,
        Z0=Z0,
        Z1=Z1,
        core_n=core_n,
        loc_n=loc_n,
        core_e=core_e,
        loc_e=loc_e,
    )


def make_weights(inputs, prep):
    """Host-folded weight tensors."""
    W1 = np.asarray(inputs["W1"], np.float32)  # [32, 64]
    b1 = np.asarray(inputs["b1"], np.float32)  # [64]
    Wpin = np.asarray(inputs["Wpin"], np.float32)  # [4, 64]
    att = np.asarray(inputs["att"], np.float32)  # [128]
    att1 = att[:C]
    att2 = att[C:]

    w1att = np.concatenate([W1 @ att1, [b1 @ att1]]).astype(np.float32)[:, None]

    R = np.vstack([W1, Wpin, b1[None, :]])  # [37, 64]
    rhs65 = np.concatenate([R, (R @ att2)[:, None]], 1)  # [37, 65]
    degsA = sorted({r[2] for r in prep["runsA"]})
    dix = {d: k for k, d in enumerate(degsA)}
    rhsdeg = np.empty((len(degsA), FP + 1, W_EF), np.float32)
    for d, k in dix.items():
        s = np.full((FP + 1, 1), 1.0 / d, np.float32)
        s[FP, 0] = 1.0  # b1 row is not divided by d
        rhsdeg[k] = rhs65 * s
    return w1att, rhsdeg, dix


# ----------------------------------------------------------- device program
def build_program(prep, inputs):
    NRN, NRE = prep["NRN"], prep["NRE"]
    nchA, nchC = prep["nchA"], prep["nchC"]
    runsA, runsC = prep["runsA"], prep["runsC"]
    TN = len(prep["tiles_n"])
    TH = len(prep["tiles_e"])
    TBN = NRN // P
    NDEG = len(sorted({r[2] for r in runsA}))
    core_ids = list(range(NCORES))
    _, _, dix = make_weights(inputs, prep)

    nc = bass.Bass("TRN2", target_bir_lowering=False, debug=False, num_devices=NCORES)

    # inputs
    hT_in = nc.declare_dram_parameter("hT", [F_IN + 1, NRN], F32, isOutput=False)
    w1att_in = nc.declare_dram_parameter("w1att", [F_IN + 1, 1], F32, isOutput=False)
    rhsdeg_in = nc.declare_dram_parameter(
        "rhsdeg", [NDEG, FP + 1, W_EF], F32, isOutput=False
    )
    hpin_in = nc.declare_dram_parameter("hpin", [nchA, P, KA * FP], BF16, isOutput=False)
    cidx0_in = nc.declare_dram_parameter("cidx0", [nchC, P, KC * 8], I16, isOutput=False)
    cidx1_in = nc.declare_dram_parameter("cidx1", [nchC, P, KC * 8], I16, isOutput=False)
    cmask_in = nc.declare_dram_parameter("cmask", [nchC, P, 2 * KC], F32, isOutput=False)
    wpool_in = nc.declare_dram_parameter("wpool", [P, TN * 2], F32, isOutput=False)
    WpostT_in = nc.declare_dram_parameter("WpostT", [4 * C, C], F32, isOutput=False)
    bpost_in = nc.declare_dram_parameter("bpost", [1, C], F32, isOutput=False)
    Wm1_in = nc.declare_dram_parameter("Wm1", [2 * C, C], F32, isOutput=False)
    bm1_in = nc.declare_dram_parameter("bm1", [1, C], F32, isOutput=False)
    Wm2_in = nc.declare_dram_parameter("Wm2", [C, C // 2], F32, isOutput=False)
    bm2_in = nc.declare_dram_parameter("bm2", [1, C // 2], F32, isOutput=False)
    Wm3_in = nc.declare_dram_parameter("Wm3", [C // 2, 1], F32, isOutput=False)
    bm3_in = nc.declare_dram_parameter("bm3", [1, 1], F32, isOutput=False)
    z_out = nc.declare_dram_parameter("z", [1, 1], F32, isOutput=True)

    # internal DRAM
    ef_shard = nc.dram_tensor("ef_shard", [NRE, 2 * C], BF16)
    ef_full = nc.dram_tensor("ef_full", [NCORES * NRE, 2 * C], BF16, addr_space="Shared")
    ar_in = nc.dram_tensor("ar_in", [2, C], F32)
    ar_out = nc.dram_tensor("ar_out", [2, C], F32, addr_space="Shared")

    u_sb = nc.alloc_sbuf_tensor("u_sb", [P, TBN], F32)  # persistent u columns

    # ---------------- context 1: phase B (u) + phase A (e_feat) ------------
    with tile.TileContext(nc) as tc:
        with (
            tc.tile_pool(name="acons", bufs=1) as acons,
            tc.tile_pool(name="bht", bufs=3) as bht,
            tc.tile_pool(name="ahp", bufs=3) as ahp,
            tc.tile_pool(name="ahsp", bufs=3) as ahsp,
            tc.tile_pool(name="atp", bufs=3) as atp,
            tc.tile_pool(name="aefb", bufs=3) as aefb,
            tc.tile_pool(name="bpsum", bufs=3, space="PSUM") as bpsum,
            tc.tile_pool(name="apsT", bufs=2, space="PSUM") as apsT,
            tc.tile_pool(name="apsE", bufs=3, space="PSUM") as apsE,
        ):
            ident = acons.tile([P, P], F32)
            make_identity(nc, ident[:])
            w1att = acons.tile([F_IN + 1, 1], F32)
            nc.sync.dma_start(out=w1att[:], in_=w1att_in[:, :])
            rhsd = acons.tile([FP + 1, NDEG * W_EF], F32)
            nc.sync.dma_start(
                out=rhsd[:].rearrange("p (n w) -> p n w", n=NDEG),
                in_=rhsdeg_in[:, :, :].rearrange("n p w -> p n w"),
            )
            # zero the dummy tile of ef_shard (window sentinel rows), all 128 ch
            zt = acons.tile([P, 2 * C], BF16)
            nc.vector.memset(zt[:], 0.0)
            nc.sync.dma_start(out=ef_shard[TH * P : (TH + 1) * P, :], in_=zt[:])

            # phase B: u = h @ (W1 att1) + b1 att1 per node tile
            GB = 16
            for g0 in range(0, TBN, GB):
                gT = min(GB, TBN - g0)
                ht = bht.tile([F_IN + 1, GB * P], F32, tag="ht")
                nc.sync.dma_start(
                    out=ht[:, : gT * P], in_=hT_in[:, g0 * P : (g0 + gT) * P]
                )
                for i in range(gT):
                    ups = bpsum.tile([P, 1], F32, space="PSUM")
                    nc.tensor.matmul(
                        ups[:],
                        lhsT=ht[:, i * P : (i + 1) * P],
                        rhs=w1att[:],
                        start=True,
                        stop=True,
                    )
                    nc.scalar.activation(
                        u_sb[:, g0 + i : g0 + i + 1], ups[:], AF.Copy
                    )

            # phase A
            run_by_chunk = {}
            for r in runsA:
                run_by_chunk.setdefault(r[0], []).append(r)
            for ch in range(nchA):
                hp = ahp.tile([P, KA * FP], BF16)
                nc.sync.dma_start(out=hp[:], in_=hpin_in[ch])
                for (_, col, d, tl) in run_by_chunk.get(ch, []):
                    T = len(tl)
                    hsp = ahsp.tile([P, MAX_RUN * FP], F32, tag="hsp")
                    nc.vector.reduce_sum(
                        hsp[:, : T * FP].rearrange("p (t f) -> p t f", t=T),
                        hp[:, col * FP : (col + T * d) * FP].rearrange(
                            "p (t j f) -> p t f j", t=T, j=d, f=FP
                        ),
                        axis=AX,
                    )
                    efb = aefb.tile([P, MAX_RUN * W_EF], BF16, tag="efb")
                    for ti in range(T):
                        tps = apsT.tile([P, P], F32, space="PSUM", tag="tps")
                        nc.tensor.transpose(
                            out=tps[:FP, :],
                            in_=hsp[:, ti * FP : (ti + 1) * FP],
                            identity=ident[:],
                        )
                        lt = atp.tile([FP + 1, P], F32, tag="lt")
                        nc.vector.memset(lt[:], 1.0)
                        nc.scalar.activation(lt[:FP, :], tps[:FP, :], AF.Copy)
                        eps = apsE.tile([P, W_EF], F32, space="PSUM", tag="eps")
                        k = dix[d]
                        nc.tensor.matmul(
                            eps[:],
                            lhsT=lt[:],
                            rhs=rhsd[:, k * W_EF : (k + 1) * W_EF],
                            start=True,
                            stop=True,
                        )
                        nc.scalar.activation(
                            efb[:, ti * W_EF : (ti + 1) * W_EF], eps[:], AF.Copy
                        )
                    t0 = tl[0]
                    nc.sync.dma_start(
                        out=ef_shard[t0 * P : (t0 + T) * P, :W_EF].rearrange(
                            "(t p) w -> p t w", p=P
                        ),
                        in_=efb[:, : T * W_EF].rearrange("p (t w) -> p t w", t=T),
                    )

    # AllGather ef
    with (
        nc.semaphore("ag2_sem") as ag2_sem,
        nc.Block() as blk2,
    ):

        @blk2.gpsimd
        def _(g):
            g.collective_compute(
                "AllGather",
                mybir.AluOpType.bypass,
                replica_groups=[core_ids],
                ins=[ef_shard[:, :]],
                outs=[ef_full[:, :]],
            ).then_inc(ag2_sem, 1)
            g.wait_ge(ag2_sem, 1)

    # ---------------- context 2: phase C (attention + PNA + pooling) -------
    NI = KC * P  # idxs per gather
    ef_pairs = ef_full[:, :].rearrange("(r two) w -> r (two w)", two=2)
    with tile.TileContext(nc) as tc:
        with (
            tc.tile_pool(name="ccons", bufs=1) as ccons,
            tc.tile_pool(name="cidxp", bufs=4) as cidxp,
            tc.tile_pool(name="cmaskp", bufs=2) as cmaskp,
            tc.tile_pool(name="cstg", bufs=4) as cstg,
            tc.tile_pool(name="cSp", bufs=1) as cSp,
            tc.tile_pool(name="csml", bufs=3) as csml,
            tc.tile_pool(name="cmsg", bufs=1) as cmsg,
            tc.tile_pool(name="cpost", bufs=1) as cpost,
            tc.tile_pool(name="chx", bufs=3) as chx,
            tc.tile_pool(name="cppsum", bufs=1, space="PSUM") as cppsum,
            tc.tile_pool(name="ctpsum", bufs=2, space="PSUM") as ctpsum,
        ):
            identC = ccons.tile([P, P], F32)
            make_identity(nc, identC[:])
            nc.gpsimd.load_library(library_config.mlp)
            wpostT_a = ccons.tile([P, C], F32)
            nc.sync.dma_start(out=wpostT_a[:], in_=WpostT_in[:P, :])
            wpostT_b = ccons.tile([P, C], F32)
            nc.sync.dma_start(out=wpostT_b[:], in_=WpostT_in[P:, :])
            bpost = ccons.tile([1, C], F32)
            nc.sync.dma_start(out=bpost[:], in_=bpost_in[:, :])
            ones1 = ccons.tile([1, P], F32)
            nc.vector.memset(ones1[:], 1.0)
            wpool = ccons.tile([P, TN * 2], F32)
            nc.sync.dma_start(out=wpool[:], in_=wpool_in[:, :])
            pool_ps = cppsum.tile([2, C], F32, space="PSUM")
            slope_c = ccons.tile([P, 1], F32)
            nc.vector.memset(slope_c[:], SLOPE)
            invd_c = {}
            for d in sorted({r[2] for r in runsC}):
                t = ccons.tile([P, 1], F32, tag=f"invd{d}")
                nc.vector.memset(t[:], 1.0 / d)
                invd_c[d] = t
            eps_c = ccons.tile([P, 1], F32)
            nc.vector.memset(eps_c[:], 1e-12)

            run_by_chunkC = {}
            for r in runsC:
                run_by_chunkC.setdefault(r[0], []).append(r)

            ni_reg = nc.gpsimd.to_reg(NI)
            first_mm = [True]
            n_tiles_done = [0]
            for ch in range(nchC):
                it0 = cidxp.tile([P, KC * 8], I16, tag="it0")
                nc.sync.dma_start(out=it0[:], in_=cidx0_in[ch])
                it1 = cidxp.tile([P, KC * 8], I16, tag="it1")
                nc.sync.dma_start(out=it1[:], in_=cidx1_in[ch])
                mk = cmaskp.tile([P, 2 * KC], F32)
                nc.sync.dma_start(out=mk[:], in_=cmask_in[ch])
                stgA = cstg.tile([P, KC * 2 * P], BF16, tag="stgA")
                nc.gpsimd.dma_gather(
                    out_ap=stgA[:].rearrange("p (c e) -> p c e", e=2 * P),
                    in_ap=ef_pairs[:, :],
                    idxs_ap=it0[:],
                    num_idxs=NI,
                    num_idxs_reg=ni_reg,
                    elem_size=2 * P,
                    single_packet=False,
                )
                stgB = cstg.tile([P, KC * 2 * P], BF16, tag="stgB")
                nc.gpsimd.dma_gather(
                    out_ap=stgB[:].rearrange("p (c e) -> p c e", e=2 * P),
                    in_ap=ef_pairs[WIN:, :],
                    idxs_ap=it1[:],
                    num_idxs=NI,
                    num_idxs_reg=ni_reg,
                    elem_size=2 * P,
                    single_packet=False,
                )
                S = cSp.tile([P, KC * 2 * P], BF16)
                nc.vector.tensor_tensor(
                    out=S[:], in0=stgA[:], in1=stgB[:], op=mybir.AluOpType.add
                )
                for (_, col, d, tl) in run_by_chunkC.get(ch, []):
                    T = len(tl)
                    F = T * d
                    t0 = tl[0]
                    S3 = S[:, col * 2 * P : (col + F) * 2 * P].rearrange(
                        "p (s e) -> p s e", s=F
                    )
                    mkE = mk[:, col : col + F]
                    mkO = mk[:, KC + col : KC + col + F]

                    # v select
                    vsel = csml.tile([P, KC], F32, tag="vsel")
                    nc.vector.tensor_tensor(
                        out=vsel[:, :F], in0=S3[:, :, C], in1=mkE,
                        op=mybir.AluOpType.mult,
                    )
                    vtmp = csml.tile([P, KC], F32, tag="vtmp")
                    nc.vector.tensor_tensor(
                        out=vtmp[:, :F], in0=S3[:, :, P + C], in1=mkO,
                        op=mybir.AluOpType.mult,
                    )
                    nc.vector.tensor_tensor(
                        out=vsel[:, :F], in0=vsel[:, :F], in1=vtmp[:, :F],
                        op=mybir.AluOpType.add,
                    )

                    # a = lrelu(u + v); ex = exp(a)  (no max-subtraction)
                    asl = csml.tile([P, KC], F32, tag="asl")
                    nc.vector.tensor_tensor(
                        out=asl[:, :F].rearrange("p (t j) -> p t j", t=T),
                        in0=u_sb[:, t0 : t0 + T, None].to_broadcast([P, T, d]),
                        in1=vsel[:, :F].rearrange("p (t j) -> p t j", t=T),
                        op=mybir.AluOpType.add,
                    )
                    a2 = csml.tile([P, KC], F32, tag="a2")
                    nc.vector.tensor_scalar_mul(a2[:, :F], asl[:, :F], SLOPE)
                    nc.vector.tensor_tensor(
                        out=asl[:, :F], in0=asl[:, :F], in1=a2[:, :F],
                        op=mybir.AluOpType.max,
                    )
                    ex = csml.tile([P, KC], F32, tag="ex")
                    nc.scalar.activation(ex[:, :F], asl[:, :F], AF.Exp)
                    den = csml.tile([P, MAX_RUN], F32, tag="den")
                    nc.vector.reduce_sum(
                        den[:, :T],
                        ex[:, :F].rearrange("p (t j) -> p t j", t=T),
                        axis=AX,
                    )
                    nc.vector.reciprocal(den[:, :T], den[:, :T])
                    alpha = csml.tile([P, KC], F32, tag="alpha")
                    nc.vector.tensor_tensor(
                        out=alpha[:, :F].rearrange("p (t j) -> p t j", t=T),
                        in0=ex[:, :F].rearrange("p (t j) -> p t j", t=T),
                        in1=den[:, :T, None].to_broadcast([P, T, d]),
                        op=mybir.AluOpType.mult,
                    )
                    alE = csml.tile([P, KC], F32, tag="alE")
                    nc.vector.tensor_tensor(
                        out=alE[:, :F], in0=alpha[:, :F], in1=mkE,
                        op=mybir.AluOpType.mult,
                    )
                    alO = csml.tile([P, KC], F32, tag="alO")
                    nc.vector.tensor_tensor(
                        out=alO[:, :F], in0=alpha[:, :F], in1=mkO,
                        op=mybir.AluOpType.mult,
                    )

                    # msg = alE * S_even + alO * S_odd
                    msg = cmsg.tile([P, KC * C], F32, tag="msg")
                    nc.vector.tensor_tensor(
                        out=msg[:, : F * C].rearrange("p (s d) -> p s d", s=F),
                        in0=S3[:, :, :C],
                        in1=alE[:, :F, None].to_broadcast([P, F, C]),
                        op=mybir.AluOpType.mult,
                    )
                    mtmp = cmsg.tile([P, KC * C], F32, tag="mtmp")
                    nc.vector.tensor_tensor(
                        out=mtmp[:, : F * C].rearrange("p (s d) -> p s d", s=F),
                        in0=S3[:, :, P : P + C],
                        in1=alO[:, :F, None].to_broadcast([P, F, C]),
                        op=mybir.AluOpType.mult,
                    )
                    nc.vector.tensor_tensor(
                        out=msg[:, : F * C], in0=msg[:, : F * C],
                        in1=mtmp[:, : F * C], op=mybir.AluOpType.add,
                    )
                    msg3 = msg[:, : F * C].rearrange(
                        "p (t j d) -> p t d j", t=T, j=d, d=C
                    )
                    # pna = [mean | mx | mn | std] per tile
                    post = cpost.tile([P, MAX_RUN * 4 * C], F32, tag="post")
                    post3 = post[:, : T * 4 * C].rearrange(
                        "p (t q d) -> p t q d", t=T, q=4
                    )
                    nc.vector.reduce_max(post3[:, :, 1], msg3, axis=AX)
                    nc.vector.tensor_reduce(
                        post3[:, :, 2], msg3, op=mybir.AluOpType.min, axis=AX
                    )
                    sm = cpost.tile([P, MAX_RUN * C], F32, tag="sm")
                    nc.vector.reduce_sum(
                        sm[:, : T * C].rearrange("p (t d) -> p t d", t=T), msg3,
                        axis=AX,
                    )
                    nc.scalar.activation(
                        post3[:, :, 0],
                        sm[:, : T * C].rearrange("p (t d) -> p t d", t=T),
                        AF.Copy,
                        scale=invd_c[d][:],
                    )
                    msq = cmsg.tile([P, KC * C], F32, tag="msq")
                    nc.scalar.activation(msq[:, : F * C], msg[:, : F * C], AF.Square)
                    sq = cpost.tile([P, MAX_RUN * C], F32, tag="sq")
                    nc.vector.reduce_sum(
                        sq[:, : T * C].rearrange("p (t d) -> p t d", t=T),
                        msq[:, : F * C].rearrange("p (t j d) -> p t d j", t=T, j=d),
                        axis=AX,
                    )
                    m2 = cpost.tile([P, MAX_RUN * C], F32, tag="m2")
                    nc.vector.tensor_tensor(
                        out=m2[:, : T * C].rearrange("p (t d) -> p t d", t=T),
                        in0=post3[:, :, 0],
                        in1=post3[:, :, 0],
                        op=mybir.AluOpType.mult,
                    )
                    nc.vector.tensor_scalar(
                        out=sq[:, : T * C], in0=sq[:, : T * C],
                        scalar1=1.0 / d, scalar2=None, op0=mybir.AluOpType.mult,
                    )
                    nc.vector.tensor_tensor(
                        out=sq[:, : T * C], in0=sq[:, : T * C],
                        in1=m2[:, : T * C], op=mybir.AluOpType.subtract,
                    )
                    nc.vector.tensor_scalar_max(sq[:, : T * C], sq[:, : T * C], 0.0)
                    nc.scalar.activation(
                        post3[:, :, 3],
                        sq[:, : T * C].rearrange("p (t d) -> p t d", t=T),
                        AF.Sqrt,
                        bias=eps_c[:],
                    )
                    # hx = lrelu(pna @ Wpost + bpost); pool matmul
                    for ti in range(T):
                        t = t0 + ti
                        pn = post[:, ti * 4 * C : (ti + 1) * 4 * C]
                        pT_ps = ctpsum.tile([P, P], F32, space="PSUM", tag="pT")
                        nc.tensor.transpose(
                            out=pT_ps[:], in_=pn[:, :P], identity=identC[:]
                        )
                        pT = chx.tile([P, 2 * P], F32, tag="pT_sb")
                        nc.scalar.activation(pT[:, :P], pT_ps[:], AF.Copy)
                        pT_ps2 = ctpsum.tile([P, P], F32, space="PSUM", tag="pT2")
                        nc.tensor.transpose(
                            out=pT_ps2[:], in_=pn[:, P:], identity=identC[:]
                        )
                        nc.scalar.activation(pT[:, P:], pT_ps2[:], AF.Copy)
                        hx_ps = ctpsum.tile([P, C], F32, space="PSUM", tag="hx")
                        nc.tensor.matmul(
                            hx_ps[:], lhsT=pT[:, :P], rhs=wpostT_a[:],
                            start=True, stop=False,
                        )
                        nc.tensor.matmul(
                            hx_ps[:], lhsT=pT[:, P:], rhs=wpostT_b[:],
                            start=False, stop=False,
                        )
                        nc.tensor.matmul(
                            hx_ps[:], lhsT=ones1[:], rhs=bpost[:],
                            start=False, stop=True,
                        )
                        hx = chx.tile([P, C], F32, tag="hx_sb")
                        hxm = chx.tile([P, C], F32, tag="hxm_sb")
                        nc.scalar.activation(
                            hxm[:], hx_ps[:], AF.Copy, scale=slope_c[:]
                        )
                        nc.vector.tensor_tensor(
                            out=hx[:], in0=hx_ps[:], in1=hxm[:],
                            op=mybir.AluOpType.max,
                        )
                        n_tiles_done[0] += 1
                        nc.tensor.matmul(
                            pool_ps[:],
                            lhsT=wpool[:, 2 * t : 2 * t + 2],
                            rhs=hx[:],
                            start=first_mm[0],
                            stop=(n_tiles_done[0] == TN),
                        )
                        first_mm[0] = False

            pool_sb = ccons.tile([2, C], F32)
            nc.vector.tensor_copy(pool_sb[:], pool_ps[:])
            nc.sync.dma_start(out=ar_in[:, :], in_=pool_sb[:])

    # AllReduce pooled partials
    with (
        nc.semaphore("ar_sem") as ar_sem,
        nc.Block() as blk3,
    ):

        @blk3.gpsimd
        def _(g):
            g.collective_compute(
                "AllReduce",
                mybir.AluOpType.add,
                replica_groups=[core_ids],
                ins=[ar_in[:, :]],
                outs=[ar_out[:, :]],
            ).then_inc(ar_sem, 1)
            g.wait_ge(ar_sem, 1)

    # ---------------- context 3: MLP head ---------------------------------
    with tile.TileContext(nc) as tc:
        with (
            tc.tile_pool(name="mpool", bufs=1) as mpool,
            tc.tile_pool(name="mpsum", bufs=1, space="PSUM") as mpsum,
        ):
            identM = mpool.tile([P, P], F32)
            make_identity(nc, identM[:])
            onesM = mpool.tile([1, 1], F32)
            nc.vector.memset(onesM[:], 1.0)
            sc_mac = mpool.tile([C, 1], F32)
            nc.vector.memset(sc_mac[:], 1.0 / 512.0)
            sc_all = mpool.tile([C, 1], F32)
            nc.vector.memset(sc_all[:], 1.0 / N_NODES)
            sl1 = mpool.tile([1, 1], F32)
            nc.vector.memset(sl1[:], SLOPE)
            pool2 = mpool.tile([2, C], F32)
            nc.sync.dma_start(out=pool2[:], in_=ar_out[:, :])
            poolT_ps = mpsum.tile([P, P], F32, space="PSUM")
            nc.tensor.transpose(
                out=poolT_ps[:C, :2], in_=pool2[:], identity=identM[:2, :2]
            )
            pooled = mpool.tile([P, 1], F32)
            nc.scalar.activation(
                pooled[:C, :], poolT_ps[:C, :1], AF.Copy, scale=sc_mac[:]
            )
            nc.scalar.activation(
                pooled[C:, :], poolT_ps[:C, 1:2], AF.Copy, scale=sc_all[:]
            )
            wm1 = mpool.tile([2 * C, C], F32)
            nc.sync.dma_start(out=wm1[:], in_=Wm1_in[:, :])
            bm1 = mpool.tile([1, C], F32)
            nc.sync.dma_start(out=bm1[:], in_=bm1_in[:, :])
            wm2 = mpool.tile([C, C // 2], F32)
            nc.sync.dma_start(out=wm2[:], in_=Wm2_in[:, :])
            bm2 = mpool.tile([1, C // 2], F32)
            nc.sync.dma_start(out=bm2[:], in_=bm2_in[:, :])
            wm3 = mpool.tile([C // 2, 1], F32)
            nc.sync.dma_start(out=wm3[:], in_=Wm3_in[:, :])
            bm3 = mpool.tile([1, 1], F32)
            nc.sync.dma_start(out=bm3[:], in_=bm3_in[:, :])

            def _lrelu_row(dst, src_ps, width):
                tmp = mpool.tile([1, width], F32, tag=f"lr{width}")
                nc.scalar.activation(tmp[:], src_ps[:], AF.Copy, scale=sl1[:])
                nc.vector.tensor_tensor(
                    out=dst[:], in0=src_ps[:], in1=tmp[:], op=mybir.AluOpType.max
                )

            z1_ps = mpsum.tile([1, C], F32, space="PSUM")
            nc.tensor.matmul(z1_ps[:], lhsT=pooled[:], rhs=wm1[:], start=True, stop=False)
            nc.tensor.matmul(
                z1_ps[:], lhsT=onesM[:].to_broadcast([1, 1]), rhs=bm1[:],
                start=False, stop=True,
            )
            z1 = mpool.tile([1, C], F32)
            _lrelu_row(z1, z1_ps, C)
            z1T_ps = mpsum.tile([P, P], F32, space="PSUM")
            nc.tensor.transpose(out=z1T_ps[:C, :1], in_=z1[:], identity=identM[:1, :1])
            z1T = mpool.tile([C, 1], F32)
            nc.vector.tensor_copy(z1T[:], z1T_ps[:C, :1])
            z2_ps = mpsum.tile([1, C // 2], F32, space="PSUM")
            nc.tensor.matmul(z2_ps[:], lhsT=z1T[:], rhs=wm2[:], start=True, stop=False)
            nc.tensor.matmul(
                z2_ps[:], lhsT=onesM[:].to_broadcast([1, 1]), rhs=bm2[:],
                start=False, stop=True,
            )
            z2 = mpool.tile([1, C // 2], F32)
            _lrelu_row(z2, z2_ps, C // 2)
            z2T_ps = mpsum.tile([P, P], F32, space="PSUM")
            nc.tensor.transpose(
                out=z2T_ps[: C // 2, :1], in_=z2[:], identity=identM[:1, :1]
            )
            z2T = mpool.tile([C // 2, 1], F32)
            nc.vector.tensor_copy(z2T[:], z2T_ps[: C // 2, :1])
            z3_ps = mpsum.tile([1, 1], F32, space="PSUM")
            nc.tensor.matmul(z3_ps[:], lhsT=z2T[:], rhs=wm3[:], start=True, stop=False)
            nc.tensor.matmul(
                z3_ps[:], lhsT=onesM[:].to_broadcast([1, 1]), rhs=bm3[:],
                start=False, stop=True,
            )
            z3 = mpool.tile([1, 1], F32)
            nc.vector.tensor_copy(z3[:], z3_ps[:])
            nc.sync.dma_start(out=z_out[:, :], in_=z3[:])

    _split_waits(nc)
    library_overlay.lower_extended_insts(nc)
    return nc


def make_in_maps(prep, inputs):
    w1att, rhsdeg, _ = make_weights(inputs, prep)
    Wpost = np.asarray(inputs["Wpost"], np.float32)
    in_maps = []
    for c in range(NCORES):
        pc = prep["per_core"][c]
        in_maps.append(
            dict(
                hT=pc["hT"],
                w1att=w1att,
                rhsdeg=rhsdeg,
                hpin=pc["hpin"],
                cidx0=pc["cidx0"],
                cidx1=pc["cidx1"],
                cmask=pc["cmask"],
                wpool=pc["wpool"],
                WpostT=Wpost,
                bpost=np.asarray(inputs["bpost"], np.float32)[None, :],
                Wm1=np.asarray(inputs["Wm1"], np.float32),
                bm1=np.asarray(inputs["bm1"], np.float32)[None, :],
                Wm2=np.asarray(inputs["Wm2"], np.float32),
                bm2=np.asarray(inputs["bm2"], np.float32)[None, :],
                Wm3=np.asarray(inputs["Wm3"], np.float32),
                bm3=np.asarray(inputs["bm3"], np.float32)[None, :],
            )
        )
    return in_maps


def _install_ntff_hook():
    """Register the NTFF profile hook trn_boot skips when antenv.axon_hooks is
    absent, so run_bass_kernel_spmd(trace=True) can report exec_time_ns."""
    import sys
    import types

    try:
        if "antenv.axon_hooks" not in sys.modules:
            import antenv

            mod = types.ModuleType("antenv.axon_hooks")
            holder = [None]
            mod.set_axon_ntff_profile_hook = lambda h: holder.__setitem__(0, h)
            mod.get_axon_ntff_profile_hook = lambda: holder[0]
            mod._holder = holder
            sys.modules["antenv.axon_hooks"] = mod
            antenv.axon_hooks = mod
        mod = sys.modules["antenv.axon_hooks"]
        if mod.get_axon_ntff_profile_hook() is None:
            from trn_agent_boot.trn_boot import _ntff_profile_via_ctypes

            mod.set_axon_ntff_profile_hook(
                _ntff_profile_via_ctypes("/opt/axon/libaxon_pjrt.so")
            )
        return mod.get_axon_ntff_profile_hook() is not None
    except Exception:
        return False


_LAST = {}


def kernel(**inputs):
    prep = preprocess(inputs)
    nc = build_program(prep, inputs)
    in_maps = make_in_maps(prep, inputs)
    trace_ok = _install_ntff_hook()
    try:
        res = run_bass_kernel_spmd(
            nc, in_maps, list(range(NCORES)), trace=trace_ok, trace_cores=[0]
        )
    except Exception:
        res = run_bass_kernel_spmd(nc, in_maps, list(range(NCORES)))
    _LAST["res"] = res
    return res.results[0]["z"].astype(np.float32)


# revision 14
# speedup vs baseline: 1.2455x; 1.0083x over previous
"""Trainium2 Bass kernel for nn_BNet (hypergraph GNN message passing), 8 cores.

v2 design (vs v1: no xl table, no xl AllGather, no per-pin INDIRECT1D)
----------------------------------------------------------------------
- Host stages per-pin raw features hpin = [h(32) | pin_feature(4)] in
  hyperedge-major slot layout (bf16).  Because e_feat is linear in the
  per-pin features, phase A computes each 128-hyperedge tile as
  (sum_j hpin_j) @ [W1;Wpin;b1]/d with one PE transpose + one matmul per
  tile; weight column 64 holds (.)@att2 so the matmul emits the full
  65-wide row (e_feat | v) at once.
- e_feat rows stored bf16, padded to 128 ch (256B rows) in a shard table;
  AllGather builds the global table (8*NRE rows, ~51.7k pairs < 65536).
- Phase C gathers per-pin e_feat rows with TWO windowed dma_gather custom
  instructions per 4096-slot chunk (int16 indices address 512B row-PAIRS
  at stride 512B; out-of-window slots read a guaranteed-zero row from the
  shard's dummy tile; windows merged with one bf16 add; the even/odd
  sub-row select is folded into the alpha multiply).
- Per-node softmax drops the max-subtraction (a_raw is O(5); exp is safe
  in fp32 and the subtraction cancels exactly in alpha).
- PNA (mean/max/min/std), Wpost, pooling, AllReduce and the MLP head are
  unchanged from v1.
"""

import ml_dtypes
import numpy as np

import bass_rust
import concourse.bass as bass
import concourse.tile as tile
from concourse import library_config, library_overlay, mybir
from concourse.bass_utils import run_bass_kernel_spmd
from concourse.masks import make_identity
from concourse.vector_clock import ScopedClock

# ----------------------------------------------------------------- constants
N_NODES = 200000
N_HE = 100000
NNZ = 1000000
F_IN = 32  # 29 + 2 + 1
FP = 36  # h(32) + pin_feature(4)
C = 64
NCORES = 8
P = 128
W_EF = 65  # e_feat row: 64 dims + v
KA = 128  # phase A chunk columns
KC = 32  # phase C chunk columns (4096 slots per chunk)
WIN = 32768  # int16 window size in pair rows
MAX_RUN = 8
SLOPE = 0.1
F32 = mybir.dt.float32
BF16 = mybir.dt.bfloat16
I16 = mybir.dt.int16
AX = mybir.AxisListType.X
AF = mybir.ActivationFunctionType
BF = ml_dtypes.bfloat16


# ------------------------------------------------------- walrus workarounds
def _patched_drain_and_barrier(self, tick_clock, wait_clock):
    nc = self.nc
    assert self.sems is not None
    handles = list(self.sems.allocated().values())
    scratch = nc.sync.sem_inc(handles[0], 0) if handles else nc.sync.drain()
    wait_clock.add_sem_waits(scratch.ins, ScopedClock({None: tick_clock.global_clock}))
    waits = list(scratch.ins.sync_info.on_wait)
    scratch.ins.sync_info = bass_rust.SyncInfo(on_wait=[], on_update=[])
    by_name = {h.name: h for h in handles}
    for w in waits:
        nc.sync.wait_ge(by_name[w.ant_name], w.wait_value)
    nc.sync.drain()
    nc.all_engine_barrier()
    popped = nc._tile_sem_poison_stack.pop()
    assert popped is self._sem_poison
    nc.clear_and_free_semaphores(handles)
    nc.all_engine_barrier()


tile.TileContext._drain_and_barrier = _patched_drain_and_barrier

_WS_CTR = [0]


def _split_waits(nc):
    """This walrus build allows at most one sync-wait per instruction; hoist
    extras onto NoOps inserted just before, same engine."""
    for fn in nc.m.functions:
        for bb in fn.blocks:
            insts = list(bb.instructions)
            new = []
            for inst in insts:
                si = inst.sync_info
                if si is not None and len(si.on_wait) > 1:
                    waits = list(si.on_wait)
                    for w in waits[:-1]:
                        _WS_CTR[0] += 1
                        new.append(
                            mybir.InstNoOp(
                                name=f"waitsplit_{_WS_CTR[0]}",
                                engine=inst.engine,
                                sync_info=mybir.SyncInfo(on_wait=[w], on_update=[]),
                                bass_nofuse=True,
                            )
                        )
                    inst.sync_info = mybir.SyncInfo(
                        on_wait=[waits[-1]], on_update=list(si.on_update)
                    )
                new.append(inst)
            bb.instructions = new


# ----------------------------------------------------------- preprocessing
def _partition_by_degree(deg, ncores):
    """Deal ids with deg>=1 round-robin per degree class across cores.

    Returns (core, local_row, tiles, n_rows); tiles is the common per-core
    tile list [(degree, base_row)]; n_rows includes one final all-dummy tile
    (guaranteed-zero rows; used as the window sentinel on the hyperedge side).
    """
    n = len(deg)
    order = np.lexsort((np.arange(n), deg))
    order = order[deg[order] >= 1]
    d_sorted = deg[order].astype(np.int64)
    change = np.nonzero(np.diff(d_sorted))[0] + 1
    starts = np.r_[0, change]
    ends = np.r_[change, len(order)]
    rank = np.arange(len(order)) - np.repeat(starts, ends - starts)
    core_of = (rank % ncores).astype(np.int32)
    lrank = rank // ncores
    tiles = []
    local = np.zeros(len(order), np.int64)
    base = 0
    for s, e in zip(starts, ends):
        d = int(d_sorted[s])
        m = int(np.ceil((e - s) / ncores))
        t_d = int(np.ceil(m / P))
        idx = slice(s, e)
        local[idx] = base + lrank[idx]
        for t in range(t_d):
            tiles.append((d, base + t * P))
        base += t_d * P
    n_rows = base + P
    core = np.full(n, -1, np.int32)
    loc = np.full(n, -1, np.int64)
    core[order] = core_of
    loc[order] = local
    return core, loc, tiles, n_rows


def _pack_chunks(tiles, kch):
    """Pack tiles into kch-slot chunks; a tile never crosses a chunk.
    Returns (placement [(chunk, col)], n_chunks, runs, used) with runs =
    [(chunk, col0, degree, [tile_indices])] capped at MAX_RUN tiles."""
    place = []
    chunk, cur = 0, 0
    used = {}
    for d, _ in tiles:
        assert d <= kch, f"degree {d} exceeds chunk width {kch}"
        if cur + d > kch:
            chunk += 1
            cur = 0
        place.append((chunk, cur))
        cur += d
        used[chunk] = cur
    n_chunks = chunk + 1
    runs = []
    i = 0
    while i < len(tiles):
        d = tiles[i][0]
        ch, col = place[i]
        j = i
        while (
            j + 1 < len(tiles)
            and tiles[j + 1][0] == d
            and place[j + 1][0] == ch
            and j + 1 - i + 1 <= MAX_RUN
        ):
            j += 1
        runs.append((ch, col, d, list(range(i, j + 1))))
        i = j + 1
    return place, n_chunks, runs, used


def _rank_within(seg_ids):
    order = np.argsort(seg_ids, kind="stable")
    sorted_ids = seg_ids[order]
    change = np.nonzero(np.diff(sorted_ids))[0] + 1
    starts = np.r_[0, change]
    counts = np.diff(np.r_[starts, len(sorted_ids)])
    r = np.arange(len(sorted_ids)) - np.repeat(starts, counts)
    out = np.empty(len(seg_ids), np.int64)
    out[order] = r
    return out


def preprocess(inputs):
    x = np.asarray(inputs["x"], np.float32)
    fake_pos = np.asarray(inputs["fake_pos"], np.float32)
    edge_index = np.asarray(inputs["edge_index"])
    pin_feature = np.asarray(inputs["pin_feature"], np.float32)
    macro_index = np.asarray(inputs["macro_index"])
    node_idx = edge_index[0].astype(np.int64)
    he_idx = edge_index[1].astype(np.int64)

    deg_n = np.bincount(node_idx, minlength=N_NODES)
    deg_e = np.bincount(he_idx, minlength=N_HE)

    core_n, loc_n, tiles_n, NRN = _partition_by_degree(deg_n, NCORES)
    core_e, loc_e, tiles_e, NRE = _partition_by_degree(deg_e, NCORES)
    placeA, nchA, runsA, usedA = _pack_chunks(tiles_e, KA)
    placeC, nchC, runsC, usedC = _pack_chunks(tiles_n, KC)

    NPAIR = NCORES * (NRE // 2)
    assert NRE % 2 == 0 and NPAIR <= 2 * WIN, (NRE, NPAIR)
    TH = len(tiles_e)
    z_pair_loc = (TH * P) // 2  # first pair row of the all-dummy (zero) tile
    Z0 = 0 * (NRE // 2) + z_pair_loc
    Z1 = (NCORES - 1) * (NRE // 2) + z_pair_loc
    assert Z0 < WIN <= Z1 and Z1 - WIN <= WIN - 1, (Z0, Z1)

    grow_pair = (core_e.astype(np.int64) * NRE + loc_e) // 2
    parity = loc_e % 2

    # per-pin placement: hyperedge-major (phase A)
    jA = _rank_within(he_idx)
    cA = core_e[he_idx]
    tA = loc_e[he_idx] // P
    pA = loc_e[he_idx] % P
    chA = np.array([pl[0] for pl in placeA], np.int64)[tA]
    colA = np.array([pl[1] for pl in placeA], np.int64)[tA] + jA

    # node-major (phase C)
    jC = _rank_within(node_idx)
    cC = core_n[node_idx]
    tC = loc_n[node_idx] // P
    pC = loc_n[node_idx] % P
    chC = np.array([pl[0] for pl in placeC], np.int64)[tC]
    colC = np.array([pl[1] for pl in placeC], np.int64)[tC] + jC

    ismacro = np.zeros(N_NODES, np.float32)
    ismacro[macro_index] = 1.0
    mult = np.bincount(macro_index, minlength=N_NODES).astype(np.float32)

    TN = len(tiles_n)
    h_full = np.concatenate([x, fake_pos, ismacro[:, None]], 1)  # (N, 32)
    hp_full = np.concatenate([h_full[node_idx], pin_feature], 1)  # (NNZ, 36)

    per_core = []
    for c in range(NCORES):
        m = cA == c
        hpin = np.zeros((nchA, P, KA, FP), np.float32)
        hpin[chA[m], pA[m], colA[m]] = hp_full[m]
        hpin_bf = np.ascontiguousarray(
            hpin.astype(BF).reshape(nchA, P, KA * FP)
        )

        # phase C windowed gather indices + parity masks
        m2 = cC == c
        gp = np.full((nchC, P, KC), Z0, np.int64)
        par = np.zeros((nchC, P, KC), np.int64)
        live = np.zeros((nchC, P, KC), bool)
        gp[chC[m2], pC[m2], colC[m2]] = grow_pair[he_idx[m2]]
        par[chC[m2], pC[m2], colC[m2]] = parity[he_idx[m2]]
        live[chC[m2], pC[m2], colC[m2]] = True

        idx0 = np.where(gp < WIN, gp, Z0)
        idx1 = np.where(gp >= WIN, gp - WIN, Z1 - WIN)

        def wrapc(a):
            # slot (p, col) -> linear i = col*128 + p -> [16, NI/16] x8 groups
            out = np.empty((nchC, P, KC * 8), np.int16)
            for ch in range(nchC):
                lin = a[ch].T.reshape(-1)  # i = col*128 + p
                m16 = lin.reshape(-1, 16).T  # [16, NI/16]
                out[ch] = np.tile(m16, (8, 1))
            return out

        cidx0 = wrapc(idx0)
        cidx1 = wrapc(idx1)
        maskE = (live & (par == 0)).astype(np.float32)
        maskO = (live & (par == 1)).astype(np.float32)
        cmask = np.ascontiguousarray(
            np.concatenate([maskE, maskO], axis=2)
        )  # [nchC, P, 2*KC] f32

        # hT (33, NRN): own nodes' features transposed + ones row
        hT = np.zeros((F_IN + 1, NRN), np.float32)
        sel = core_n == c
        hT[:F_IN, loc_n[sel]] = h_full[sel].T
        hT[F_IN, loc_n[sel]] = 1.0

        # pooling weights [128, TN*2]
        wpool = np.zeros((P, TN * 2), np.float32)
        nl = loc_n[sel]
        wpool[nl % P, (nl // P) * 2] = mult[sel]
        wpool[nl % P, (nl // P) * 2 + 1] = 1.0

        per_core.append(
            dict(
                hpin=hpin_bf,
                cidx0=cidx0,
                cidx1=cidx1,
                cmask=cmask,
                hT=hT,
                wpool=wpool,
                dbg=dict(gp=gp, par=par, live=live),
            )
        )

    return dict(
        per_core=per_core,
        tiles_n=tiles_n,
        tiles_e=tiles_e,
        runsA=runsA,
        runsC=runsC,
        usedA=usedA,
        usedC=usedC,
        nchA=nchA,
        nchC=nchC,
        NRN=NRN,
        NRE=NRE,
        NPAIR=NPAIR,
        Z0=Z0,
        Z1=Z1,
        core_n=core_n,
        loc_n=loc_n,
        core_e=core_e,
        loc_e=loc_e,
    )


def make_weights(inputs, prep):
    """Host-folded weight tensors."""
    W1 = np.asarray(inputs["W1"], np.float32)  # [32, 64]
    b1 = np.asarray(inputs["b1"], np.float32)  # [64]
    Wpin = np.asarray(inputs["Wpin"], np.float32)  # [4, 64]
    att = np.asarray(inputs["att"], np.float32)  # [128]
    att1 = att[:C]
    att2 = att[C:]

    w1att = np.concatenate([W1 @ att1, [b1 @ att1]]).astype(np.float32)[:, None]

    R = np.vstack([W1, Wpin, b1[None, :]])  # [37, 64]
    rhs65 = np.concatenate([R, (R @ att2)[:, None]], 1)  # [37, 65]
    degsA = sorted({r[2] for r in prep["runsA"]})
    dix = {d: k for k, d in enumerate(degsA)}
    rhsdeg = np.empty((len(degsA), FP + 1, W_EF), np.float32)
    for d, k in dix.items():
        s = np.full((FP + 1, 1), 1.0 / d, np.float32)
        s[FP, 0] = 1.0  # b1 row is not divided by d
        rhsdeg[k] = rhs65 * s
    return w1att, rhsdeg, dix


# ----------------------------------------------------------- device program
def build_program(prep, inputs):
    NRN, NRE = prep["NRN"], prep["NRE"]
    nchA, nchC = prep["nchA"], prep["nchC"]
    runsA, runsC = prep["runsA"], prep["runsC"]
    TN = len(prep["tiles_n"])
    TH = len(prep["tiles_e"])
    TBN = NRN // P
    NDEG = len(sorted({r[2] for r in runsA}))
    core_ids = list(range(NCORES))
    _, _, dix = make_weights(inputs, prep)

    nc = bass.Bass(
        "TRN2",
        target_bir_lowering=False,
        debug=False,
        num_devices=NCORES,
        num_swdge_queues=4,
    )

    # inputs
    hT_in = nc.declare_dram_parameter("hT", [F_IN + 1, NRN], F32, isOutput=False)
    w1att_in = nc.declare_dram_parameter("w1att", [F_IN + 1, 1], F32, isOutput=False)
    rhsdeg_in = nc.declare_dram_parameter(
        "rhsdeg", [NDEG, FP + 1, W_EF], F32, isOutput=False
    )
    hpin_in = nc.declare_dram_parameter("hpin", [nchA, P, KA * FP], BF16, isOutput=False)
    cidx0_in = nc.declare_dram_parameter("cidx0", [nchC, P, KC * 8], I16, isOutput=False)
    cidx1_in = nc.declare_dram_parameter("cidx1", [nchC, P, KC * 8], I16, isOutput=False)
    cmask_in = nc.declare_dram_parameter("cmask", [nchC, P, 2 * KC], F32, isOutput=False)
    wpool_in = nc.declare_dram_parameter("wpool", [P, TN * 2], F32, isOutput=False)
    WpostT_in = nc.declare_dram_parameter("WpostT", [4 * C, C], F32, isOutput=False)
    bpost_in = nc.declare_dram_parameter("bpost", [1, C], F32, isOutput=False)
    Wm1_in = nc.declare_dram_parameter("Wm1", [2 * C, C], F32, isOutput=False)
    bm1_in = nc.declare_dram_parameter("bm1", [1, C], F32, isOutput=False)
    Wm2_in = nc.declare_dram_parameter("Wm2", [C, C // 2], F32, isOutput=False)
    bm2_in = nc.declare_dram_parameter("bm2", [1, C // 2], F32, isOutput=False)
    Wm3_in = nc.declare_dram_parameter("Wm3", [C // 2, 1], F32, isOutput=False)
    bm3_in = nc.declare_dram_parameter("bm3", [1, 1], F32, isOutput=False)
    z_out = nc.declare_dram_parameter("z", [1, 1], F32, isOutput=True)

    # internal DRAM
    ef_shard = nc.dram_tensor("ef_shard", [NRE, 2 * C], BF16)
    ef_full = nc.dram_tensor("ef_full", [NCORES * NRE, 2 * C], BF16, addr_space="Shared")
    ar_in = nc.dram_tensor("ar_in", [2, C], F32)
    ar_out = nc.dram_tensor("ar_out", [2, C], F32, addr_space="Shared")

    u_sb = nc.alloc_sbuf_tensor("u_sb", [P, TBN], F32)  # persistent u columns

    # ---------------- context 1: phase B (u) + phase A (e_feat) ------------
    with tile.TileContext(nc) as tc:
        with (
            tc.tile_pool(name="acons", bufs=1) as acons,
            tc.tile_pool(name="bht", bufs=3) as bht,
            tc.tile_pool(name="ahp", bufs=3) as ahp,
            tc.tile_pool(name="ahsp", bufs=3) as ahsp,
            tc.tile_pool(name="atp", bufs=3) as atp,
            tc.tile_pool(name="aefb", bufs=3) as aefb,
            tc.tile_pool(name="bpsum", bufs=3, space="PSUM") as bpsum,
            tc.tile_pool(name="apsT", bufs=2, space="PSUM") as apsT,
            tc.tile_pool(name="apsE", bufs=3, space="PSUM") as apsE,
        ):
            ident = acons.tile([P, P], F32)
            make_identity(nc, ident[:])
            w1att = acons.tile([F_IN + 1, 1], F32)
            nc.sync.dma_start(out=w1att[:], in_=w1att_in[:, :])
            rhsd = acons.tile([FP + 1, NDEG * W_EF], F32)
            nc.sync.dma_start(
                out=rhsd[:].rearrange("p (n w) -> p n w", n=NDEG),
                in_=rhsdeg_in[:, :, :].rearrange("n p w -> p n w"),
            )
            # zero the dummy tile of ef_shard (window sentinel rows), all 128 ch
            zt = acons.tile([P, 2 * C], BF16)
            nc.vector.memset(zt[:], 0.0)
            nc.sync.dma_start(out=ef_shard[TH * P : (TH + 1) * P, :], in_=zt[:])

            # phase B: u = h @ (W1 att1) + b1 att1 per node tile
            GB = 16
            for g0 in range(0, TBN, GB):
                gT = min(GB, TBN - g0)
                ht = bht.tile([F_IN + 1, GB * P], F32, tag="ht")
                nc.sync.dma_start(
                    out=ht[:, : gT * P], in_=hT_in[:, g0 * P : (g0 + gT) * P]
                )
                for i in range(gT):
                    ups = bpsum.tile([P, 1], F32, space="PSUM")
                    nc.tensor.matmul(
                        ups[:],
                        lhsT=ht[:, i * P : (i + 1) * P],
                        rhs=w1att[:],
                        start=True,
                        stop=True,
                    )
                    nc.scalar.activation(
                        u_sb[:, g0 + i : g0 + i + 1], ups[:], AF.Copy
                    )

            # phase A
            run_by_chunk = {}
            for r in runsA:
                run_by_chunk.setdefault(r[0], []).append(r)
            for ch in range(nchA):
                hp = ahp.tile([P, KA * FP], BF16)
                nc.sync.dma_start(out=hp[:], in_=hpin_in[ch])
                for (_, col, d, tl) in run_by_chunk.get(ch, []):
                    T = len(tl)
                    hsp = ahsp.tile([P, MAX_RUN * FP], F32, tag="hsp")
                    nc.vector.reduce_sum(
                        hsp[:, : T * FP].rearrange("p (t f) -> p t f", t=T),
                        hp[:, col * FP : (col + T * d) * FP].rearrange(
                            "p (t j f) -> p t f j", t=T, j=d, f=FP
                        ),
                        axis=AX,
                    )
                    efb = aefb.tile([P, MAX_RUN * W_EF], BF16, tag="efb")
                    for ti in range(T):
                        tps = apsT.tile([P, P], F32, space="PSUM", tag="tps")
                        nc.tensor.transpose(
                            out=tps[:FP, :],
                            in_=hsp[:, ti * FP : (ti + 1) * FP],
                            identity=ident[:],
                        )
                        lt = atp.tile([FP + 1, P], F32, tag="lt")
                        nc.vector.memset(lt[:], 1.0)
                        nc.scalar.activation(lt[:FP, :], tps[:FP, :], AF.Copy)
                        eps = apsE.tile([P, W_EF], F32, space="PSUM", tag="eps")
                        k = dix[d]
                        nc.tensor.matmul(
                            eps[:],
                            lhsT=lt[:],
                            rhs=rhsd[:, k * W_EF : (k + 1) * W_EF],
                            start=True,
                            stop=True,
                        )
                        nc.scalar.activation(
                            efb[:, ti * W_EF : (ti + 1) * W_EF], eps[:], AF.Copy
                        )
                    t0 = tl[0]
                    nc.sync.dma_start(
                        out=ef_shard[t0 * P : (t0 + T) * P, :W_EF].rearrange(
                            "(t p) w -> p t w", p=P
                        ),
                        in_=efb[:, : T * W_EF].rearrange("p (t w) -> p t w", t=T),
                    )

    # AllGather ef
    with (
        nc.semaphore("ag2_sem") as ag2_sem,
        nc.Block() as blk2,
    ):

        @blk2.gpsimd
        def _(g):
            g.collective_compute(
                "AllGather",
                mybir.AluOpType.bypass,
                replica_groups=[core_ids],
                ins=[ef_shard[:, :]],
                outs=[ef_full[:, :]],
            ).then_inc(ag2_sem, 1)
            g.wait_ge(ag2_sem, 1)

    # ---------------- context 2: phase C (attention + PNA + pooling) -------
    NI = KC * P  # idxs per gather
    ef_pairs = ef_full[:, :].rearrange("(r two) w -> r (two w)", two=2)
    with tile.TileContext(nc) as tc:
        with (
            tc.tile_pool(name="ccons", bufs=1) as ccons,
            tc.tile_pool(name="cidxp", bufs=4) as cidxp,
            tc.tile_pool(name="cmaskp", bufs=2) as cmaskp,
            tc.tile_pool(name="cstg", bufs=4) as cstg,
            tc.tile_pool(name="cSp", bufs=1) as cSp,
            tc.tile_pool(name="csml", bufs=3) as csml,
            tc.tile_pool(name="cmsg", bufs=1) as cmsg,
            tc.tile_pool(name="cpost", bufs=1) as cpost,
            tc.tile_pool(name="chx", bufs=3) as chx,
            tc.tile_pool(name="cppsum", bufs=1, space="PSUM") as cppsum,
            tc.tile_pool(name="ctpsum", bufs=2, space="PSUM") as ctpsum,
        ):
            identC = ccons.tile([P, P], F32)
            make_identity(nc, identC[:])
            nc.gpsimd.load_library(library_config.mlp)
            wpostT_a = ccons.tile([P, C], F32)
            nc.sync.dma_start(out=wpostT_a[:], in_=WpostT_in[:P, :])
            wpostT_b = ccons.tile([P, C], F32)
            nc.sync.dma_start(out=wpostT_b[:], in_=WpostT_in[P:, :])
            bpost = ccons.tile([1, C], F32)
            nc.sync.dma_start(out=bpost[:], in_=bpost_in[:, :])
            ones1 = ccons.tile([1, P], F32)
            nc.vector.memset(ones1[:], 1.0)
            wpool = ccons.tile([P, TN * 2], F32)
            nc.sync.dma_start(out=wpool[:], in_=wpool_in[:, :])
            pool_ps = cppsum.tile([2, C], F32, space="PSUM")
            slope_c = ccons.tile([P, 1], F32)
            nc.vector.memset(slope_c[:], SLOPE)
            invd_c = {}
            for d in sorted({r[2] for r in runsC}):
                t = ccons.tile([P, 1], F32, tag=f"invd{d}")
                nc.vector.memset(t[:], 1.0 / d)
                invd_c[d] = t
            eps_c = ccons.tile([P, 1], F32)
            nc.vector.memset(eps_c[:], 1e-12)

            run_by_chunkC = {}
            for r in runsC:
                run_by_chunkC.setdefault(r[0], []).append(r)

            ni_reg = nc.gpsimd.to_reg(NI)
            first_mm = [True]
            n_tiles_done = [0]
            for ch in range(nchC):
                it0 = cidxp.tile([P, KC * 8], I16, tag="it0")
                nc.sync.dma_start(out=it0[:], in_=cidx0_in[ch])
                it1 = cidxp.tile([P, KC * 8], I16, tag="it1")
                nc.sync.dma_start(out=it1[:], in_=cidx1_in[ch])
                mk = cmaskp.tile([P, 2 * KC], F32)
                nc.sync.dma_start(out=mk[:], in_=cmask_in[ch])
                stgA = cstg.tile([P, KC * 2 * P], BF16, tag="stgA")
                nc.gpsimd.dma_gather(
                    out_ap=stgA[:].rearrange("p (c e) -> p c e", e=2 * P),
                    in_ap=ef_pairs[:, :],
                    idxs_ap=it0[:],
                    num_idxs=NI,
                    num_idxs_reg=ni_reg,
                    elem_size=2 * P,
                    single_packet=False,
                    queue_num=(2 * ch) % 4,
                )
                stgB = cstg.tile([P, KC * 2 * P], BF16, tag="stgB")
                nc.gpsimd.dma_gather(
                    out_ap=stgB[:].rearrange("p (c e) -> p c e", e=2 * P),
                    in_ap=ef_pairs[WIN:, :],
                    idxs_ap=it1[:],
                    num_idxs=NI,
                    num_idxs_reg=ni_reg,
                    elem_size=2 * P,
                    single_packet=False,
                    queue_num=(2 * ch + 1) % 4,
                )
                S = cSp.tile([P, KC * 2 * P], BF16)
                nc.vector.tensor_tensor(
                    out=S[:], in0=stgA[:], in1=stgB[:], op=mybir.AluOpType.add
                )
                for (_, col, d, tl) in run_by_chunkC.get(ch, []):
                    T = len(tl)
                    F = T * d
                    t0 = tl[0]
                    S3 = S[:, col * 2 * P : (col + F) * 2 * P].rearrange(
                        "p (s e) -> p s e", s=F
                    )
                    mkE = mk[:, col : col + F]
                    mkO = mk[:, KC + col : KC + col + F]

                    # v select
                    vsel = csml.tile([P, KC], F32, tag="vsel")
                    nc.vector.tensor_tensor(
                        out=vsel[:, :F], in0=S3[:, :, C], in1=mkE,
                        op=mybir.AluOpType.mult,
                    )
                    vtmp = csml.tile([P, KC], F32, tag="vtmp")
                    nc.vector.tensor_tensor(
                        out=vtmp[:, :F], in0=S3[:, :, P + C], in1=mkO,
                        op=mybir.AluOpType.mult,
                    )
                    nc.vector.tensor_tensor(
                        out=vsel[:, :F], in0=vsel[:, :F], in1=vtmp[:, :F],
                        op=mybir.AluOpType.add,
                    )

                    # a = lrelu(u + v); ex = exp(a)  (no max-subtraction)
                    asl = csml.tile([P, KC], F32, tag="asl")
                    nc.vector.tensor_tensor(
                        out=asl[:, :F].rearrange("p (t j) -> p t j", t=T),
                        in0=u_sb[:, t0 : t0 + T, None].to_broadcast([P, T, d]),
                        in1=vsel[:, :F].rearrange("p (t j) -> p t j", t=T),
                        op=mybir.AluOpType.add,
                    )
                    a2 = csml.tile([P, KC], F32, tag="a2")
                    nc.vector.tensor_scalar_mul(a2[:, :F], asl[:, :F], SLOPE)
                    nc.vector.tensor_tensor(
                        out=asl[:, :F], in0=asl[:, :F], in1=a2[:, :F],
                        op=mybir.AluOpType.max,
                    )
                    ex = csml.tile([P, KC], F32, tag="ex")
                    nc.scalar.activation(ex[:, :F], asl[:, :F], AF.Exp)
                    den = csml.tile([P, MAX_RUN], F32, tag="den")
                    nc.vector.reduce_sum(
                        den[:, :T],
                        ex[:, :F].rearrange("p (t j) -> p t j", t=T),
                        axis=AX,
                    )
                    nc.vector.reciprocal(den[:, :T], den[:, :T])
                    alpha = csml.tile([P, KC], F32, tag="alpha")
                    nc.vector.tensor_tensor(
                        out=alpha[:, :F].rearrange("p (t j) -> p t j", t=T),
                        in0=ex[:, :F].rearrange("p (t j) -> p t j", t=T),
                        in1=den[:, :T, None].to_broadcast([P, T, d]),
                        op=mybir.AluOpType.mult,
                    )
                    alE = csml.tile([P, KC], F32, tag="alE")
                    nc.vector.tensor_tensor(
                        out=alE[:, :F], in0=alpha[:, :F], in1=mkE,
                        op=mybir.AluOpType.mult,
                    )
                    alO = csml.tile([P, KC], F32, tag="alO")
                    nc.vector.tensor_tensor(
                        out=alO[:, :F], in0=alpha[:, :F], in1=mkO,
                        op=mybir.AluOpType.mult,
                    )

                    # msg = alE * S_even + alO * S_odd
                    msg = cmsg.tile([P, KC * C], F32, tag="msg")
                    nc.vector.tensor_tensor(
                        out=msg[:, : F * C].rearrange("p (s d) -> p s d", s=F),
                        in0=S3[:, :, :C],
                        in1=alE[:, :F, None].to_broadcast([P, F, C]),
                        op=mybir.AluOpType.mult,
                    )
                    mtmp = cmsg.tile([P, KC * C], F32, tag="mtmp")
                    nc.vector.tensor_tensor(
                        out=mtmp[:, : F * C].rearrange("p (s d) -> p s d", s=F),
                        in0=S3[:, :, P : P + C],
                        in1=alO[:, :F, None].to_broadcast([P, F, C]),
                        op=mybir.AluOpType.mult,
                    )
                    nc.vector.tensor_tensor(
                        out=msg[:, : F * C], in0=msg[:, : F * C],
                        in1=mtmp[:, : F * C], op=mybir.AluOpType.add,
                    )
                    msg3 = msg[:, : F * C].rearrange(
                        "p (t j d) -> p t d j", t=T, j=d, d=C
                    )
                    # pna = [mean | mx | mn | std] per tile
                    post = cpost.tile([P, MAX_RUN * 4 * C], F32, tag="post")
                    post3 = post[:, : T * 4 * C].rearrange(
                        "p (t q d) -> p t q d", t=T, q=4
                    )
                    nc.vector.reduce_max(post3[:, :, 1], msg3, axis=AX)
                    nc.vector.tensor_reduce(
                        post3[:, :, 2], msg3, op=mybir.AluOpType.min, axis=AX
                    )
                    sm = cpost.tile([P, MAX_RUN * C], F32, tag="sm")
                    nc.vector.reduce_sum(
                        sm[:, : T * C].rearrange("p (t d) -> p t d", t=T), msg3,
                        axis=AX,
                    )
                    nc.scalar.activation(
                        post3[:, :, 0],
                        sm[:, : T * C].rearrange("p (t d) -> p t d", t=T),
                        AF.Copy,
                        scale=invd_c[d][:],
                    )
                    msq = cmsg.tile([P, KC * C], F32, tag="msq")
                    nc.scalar.activation(msq[:, : F * C], msg[:, : F * C], AF.Square)
                    sq = cpost.tile([P, MAX_RUN * C], F32, tag="sq")
                    nc.vector.reduce_sum(
                        sq[:, : T * C].rearrange("p (t d) -> p t d", t=T),
                        msq[:, : F * C].rearrange("p (t j d) -> p t d j", t=T, j=d),
                        axis=AX,
                    )
                    m2 = cpost.tile([P, MAX_RUN * C], F32, tag="m2")
                    nc.vector.tensor_tensor(
                        out=m2[:, : T * C].rearrange("p (t d) -> p t d", t=T),
                        in0=post3[:, :, 0],
                        in1=post3[:, :, 0],
                        op=mybir.AluOpType.mult,
                    )
                    nc.vector.tensor_scalar(
                        out=sq[:, : T * C], in0=sq[:, : T * C],
                        scalar1=1.0 / d, scalar2=None, op0=mybir.AluOpType.mult,
                    )
                    nc.vector.tensor_tensor(
                        out=sq[:, : T * C], in0=sq[:, : T * C],
                        in1=m2[:, : T * C], op=mybir.AluOpType.subtract,
                    )
                    nc.vector.tensor_scalar_max(sq[:, : T * C], sq[:, : T * C], 0.0)
                    nc.scalar.activation(
                        post3[:, :, 3],
                        sq[:, : T * C].rearrange("p (t d) -> p t d", t=T),
                        AF.Sqrt,
                        bias=eps_c[:],
                    )
                    # hx = lrelu(pna @ Wpost + bpost); pool matmul
                    for ti in range(T):
                        t = t0 + ti
                        pn = post[:, ti * 4 * C : (ti + 1) * 4 * C]
                        pT_ps = ctpsum.tile([P, P], F32, space="PSUM", tag="pT")
                        nc.tensor.transpose(
                            out=pT_ps[:], in_=pn[:, :P], identity=identC[:]
                        )
                        pT = chx.tile([P, 2 * P], F32, tag="pT_sb")
                        nc.scalar.activation(pT[:, :P], pT_ps[:], AF.Copy)
                        pT_ps2 = ctpsum.tile([P, P], F32, space="PSUM", tag="pT2")
                        nc.tensor.transpose(
                            out=pT_ps2[:], in_=pn[:, P:], identity=identC[:]
                        )
                        nc.scalar.activation(pT[:, P:], pT_ps2[:], AF.Copy)
                        hx_ps = ctpsum.tile([P, C], F32, space="PSUM", tag="hx")
                        nc.tensor.matmul(
                            hx_ps[:], lhsT=pT[:, :P], rhs=wpostT_a[:],
                            start=True, stop=False,
                        )
                        nc.tensor.matmul(
                            hx_ps[:], lhsT=pT[:, P:], rhs=wpostT_b[:],
                            start=False, stop=False,
                        )
                        nc.tensor.matmul(
                            hx_ps[:], lhsT=ones1[:], rhs=bpost[:],
                            start=False, stop=True,
                        )
                        hx = chx.tile([P, C], F32, tag="hx_sb")
                        hxm = chx.tile([P, C], F32, tag="hxm_sb")
                        nc.scalar.activation(
                            hxm[:], hx_ps[:], AF.Copy, scale=slope_c[:]
                        )
                        nc.vector.tensor_tensor(
                            out=hx[:], in0=hx_ps[:], in1=hxm[:],
                            op=mybir.AluOpType.max,
                        )
                        n_tiles_done[0] += 1
                        nc.tensor.matmul(
                            pool_ps[:],
                            lhsT=wpool[:, 2 * t : 2 * t + 2],
                            rhs=hx[:],
                            start=first_mm[0],
                            stop=(n_tiles_done[0] == TN),
                        )
                        first_mm[0] = False

            pool_sb = ccons.tile([2, C], F32)
            nc.vector.tensor_copy(pool_sb[:], pool_ps[:])
            nc.sync.dma_start(out=ar_in[:, :], in_=pool_sb[:])

    # AllReduce pooled partials
    with (
        nc.semaphore("ar_sem") as ar_sem,
        nc.Block() as blk3,
    ):

        @blk3.gpsimd
        def _(g):
            g.collective_compute(
                "AllReduce",
                mybir.AluOpType.add,
                replica_groups=[core_ids],
                ins=[ar_in[:, :]],
                outs=[ar_out[:, :]],
            ).then_inc(ar_sem, 1)
            g.wait_ge(ar_sem, 1)

    # ---------------- context 3: MLP head ---------------------------------
    with tile.TileContext(nc) as tc:
        with (
            tc.tile_pool(name="mpool", bufs=1) as mpool,
            tc.tile_pool(name="mpsum", bufs=1, space="PSUM") as mpsum,
        ):
            identM = mpool.tile([P, P], F32)
            make_identity(nc, identM[:])
            onesM = mpool.tile([1, 1], F32)
            nc.vector.memset(onesM[:], 1.0)
            sc_mac = mpool.tile([C, 1], F32)
            nc.vector.memset(sc_mac[:], 1.0 / 512.0)
            sc_all = mpool.tile([C, 1], F32)
            nc.vector.memset(sc_all[:], 1.0 / N_NODES)
            sl1 = mpool.tile([1, 1], F32)
            nc.vector.memset(sl1[:], SLOPE)
            pool2 = mpool.tile([2, C], F32)
            nc.sync.dma_start(out=pool2[:], in_=ar_out[:, :])
            poolT_ps = mpsum.tile([P, P], F32, space="PSUM")
            nc.tensor.transpose(
                out=poolT_ps[:C, :2], in_=pool2[:], identity=identM[:2, :2]
            )
            pooled = mpool.tile([P, 1], F32)
            nc.scalar.activation(
                pooled[:C, :], poolT_ps[:C, :1], AF.Copy, scale=sc_mac[:]
            )
            nc.scalar.activation(
                pooled[C:, :], poolT_ps[:C, 1:2], AF.Copy, scale=sc_all[:]
            )
            wm1 = mpool.tile([2 * C, C], F32)
            nc.sync.dma_start(out=wm1[:], in_=Wm1_in[:, :])
            bm1 = mpool.tile([1, C], F32)
            nc.sync.dma_start(out=bm1[:], in_=bm1_in[:, :])
            wm2 = mpool.tile([C, C // 2], F32)
            nc.sync.dma_start(out=wm2[:], in_=Wm2_in[:, :])
            bm2 = mpool.tile([1, C // 2], F32)
            nc.sync.dma_start(out=bm2[:], in_=bm2_in[:, :])
            wm3 = mpool.tile([C // 2, 1], F32)
            nc.sync.dma_start(out=wm3[:], in_=Wm3_in[:, :])
            bm3 = mpool.tile([1, 1], F32)
            nc.sync.dma_start(out=bm3[:], in_=bm3_in[:, :])

            def _lrelu_row(dst, src_ps, width):
                tmp = mpool.tile([1, width], F32, tag=f"lr{width}")
                nc.scalar.activation(tmp[:], src_ps[:], AF.Copy, scale=sl1[:])
                nc.vector.tensor_tensor(
                    out=dst[:], in0=src_ps[:], in1=tmp[:], op=mybir.AluOpType.max
                )

            z1_ps = mpsum.tile([1, C], F32, space="PSUM")
            nc.tensor.matmul(z1_ps[:], lhsT=pooled[:], rhs=wm1[:], start=True, stop=False)
            nc.tensor.matmul(
                z1_ps[:], lhsT=onesM[:].to_broadcast([1, 1]), rhs=bm1[:],
                start=False, stop=True,
            )
            z1 = mpool.tile([1, C], F32)
            _lrelu_row(z1, z1_ps, C)
            z1T_ps = mpsum.tile([P, P], F32, space="PSUM")
            nc.tensor.transpose(out=z1T_ps[:C, :1], in_=z1[:], identity=identM[:1, :1])
            z1T = mpool.tile([C, 1], F32)
            nc.vector.tensor_copy(z1T[:], z1T_ps[:C, :1])
            z2_ps = mpsum.tile([1, C // 2], F32, space="PSUM")
            nc.tensor.matmul(z2_ps[:], lhsT=z1T[:], rhs=wm2[:], start=True, stop=False)
            nc.tensor.matmul(
                z2_ps[:], lhsT=onesM[:].to_broadcast([1, 1]), rhs=bm2[:],
                start=False, stop=True,
            )
            z2 = mpool.tile([1, C // 2], F32)
            _lrelu_row(z2, z2_ps, C // 2)
            z2T_ps = mpsum.tile([P, P], F32, space="PSUM")
            nc.tensor.transpose(
                out=z2T_ps[: C // 2, :1], in_=z2[:], identity=identM[:1, :1]
            )
            z2T = mpool.tile([C // 2, 1], F32)
            nc.vector.tensor_copy(z2T[:], z2T_ps[: C // 2, :1])
            z3_ps = mpsum.tile([1, 1], F32, space="PSUM")
            nc.tensor.matmul(z3_ps[:], lhsT=z2T[:], rhs=wm3[:], start=True, stop=False)
            nc.tensor.matmul(
                z3_ps[:], lhsT=onesM[:].to_broadcast([1, 1]), rhs=bm3[:],
                start=False, stop=True,
            )
            z3 = mpool.tile([1, 1], F32)
            nc.vector.tensor_copy(z3[:], z3_ps[:])
            nc.sync.dma_start(out=z_out[:, :], in_=z3[:])

    _split_waits(nc)
    library_overlay.lower_extended_insts(nc)
    return nc


def make_in_maps(prep, inputs):
    w1att, rhsdeg, _ = make_weights(inputs, prep)
    Wpost = np.asarray(inputs["Wpost"], np.float32)
    in_maps = []
    for c in range(NCORES):
        pc = prep["per_core"][c]
        in_maps.append(
            dict(
                hT=pc["hT"],
                w1att=w1att,
                rhsdeg=rhsdeg,
                hpin=pc["hpin"],
                cidx0=pc["cidx0"],
                cidx1=pc["cidx1"],
                cmask=pc["cmask"],
                wpool=pc["wpool"],
                WpostT=Wpost,
                bpost=np.asarray(inputs["bpost"], np.float32)[None, :],
                Wm1=np.asarray(inputs["Wm1"], np.float32),
                bm1=np.asarray(inputs["bm1"], np.float32)[None, :],
                Wm2=np.asarray(inputs["Wm2"], np.float32),
                bm2=np.asarray(inputs["bm2"], np.float32)[None, :],
                Wm3=np.asarray(inputs["Wm3"], np.float32),
                bm3=np.asarray(inputs["bm3"], np.float32)[None, :],
            )
        )
    return in_maps


def _install_ntff_hook():
    """Register the NTFF profile hook trn_boot skips when antenv.axon_hooks is
    absent, so run_bass_kernel_spmd(trace=True) can report exec_time_ns."""
    import sys
    import types

    try:
        if "antenv.axon_hooks" not in sys.modules:
            import antenv

            mod = types.ModuleType("antenv.axon_hooks")
            holder = [None]
            mod.set_axon_ntff_profile_hook = lambda h: holder.__setitem__(0, h)
            mod.get_axon_ntff_profile_hook = lambda: holder[0]
            mod._holder = holder
            sys.modules["antenv.axon_hooks"] = mod
            antenv.axon_hooks = mod
        mod = sys.modules["antenv.axon_hooks"]
        if mod.get_axon_ntff_profile_hook() is None:
            from trn_agent_boot.trn_boot import _ntff_profile_via_ctypes

            mod.set_axon_ntff_profile_hook(
                _ntff_profile_via_ctypes("/opt/axon/libaxon_pjrt.so")
            )
        return mod.get_axon_ntff_profile_hook() is not None
    except Exception:
        return False


_LAST = {}


def kernel(**inputs):
    prep = preprocess(inputs)
    nc = build_program(prep, inputs)
    in_maps = make_in_maps(prep, inputs)
    trace_ok = _install_ntff_hook()
    try:
        res = run_bass_kernel_spmd(
            nc, in_maps, list(range(NCORES)), trace=trace_ok, trace_cores=[0]
        )
    except Exception:
        res = run_bass_kernel_spmd(nc, in_maps, list(range(NCORES)))
    _LAST["res"] = res
    return res.results[0]["z"].astype(np.float32)
